# revision 26
# baseline (speedup 1.0000x reference)
"""Trainium2 Bass kernel for nn_DecoderLayer (B=2,T=2048,D=1024,H=16,dk=dv=64,dff=4096).

Sharding: 8 cores = 2 batch groups (data parallel) x 4 ranks; rank c owns
contiguous rows [512c, 512c+512) of its batch.
  - Self-attention: head-parallel (4 heads/core); S^T=[k,q] blocks; softmax
    denominator fused into AV as a ones-column of V; single AV matmul with
    full 128-key contraction. The cross-attention V projection (enc @ Wv,
    all 16 heads) is interleaved into self-attention as tensor-engine
    filler so the PE busy-streak (pstate) never drops.
  - Wo1 partials -> ONE bf16 ReduceScatter(add), hidden under the full-head
    cross K projection.
  - Cross-attention is Q-SHARDED: every rank computes all 16 heads for its
    own 512 rows, so Q2 needs only local LN1 output (no AllGather) and the
    Wo2 contraction is fully local (no second ReduceScatter).
  - FFN row-sharded, no collective. LayerNorms on own rows.
All biases are folded host-side: v-bias/Wo-bias via softmax sum-to-1 into
the LN residuals (x_rows, ln1_b, ln2_b) with Q-bias/FFN-b1 compensations.
Nearly all matmuls run in bf16 (error budget 2e-2, achieved ~1.6e-3);
residual/LN arithmetic stays fp32.
"""
from contextlib import ExitStack

import numpy as np

import concourse.bacc as bacc
import concourse.tile as tile
import concourse.mybir as mybir
from concourse.bass_utils import run_bass_kernel_spmd
from concourse.masks import make_identity

F32 = mybir.dt.float32
F32R = mybir.dt.float32r
BF16 = mybir.dt.bfloat16
AF = mybir.ActivationFunctionType
ALU = mybir.AluOpType
P = 128

B, T, D, H, DK, DV, DFF = 2, 2048, 1024, 16, 64, 64, 4096
NC, TPG = 8, 4
TOWN = T // TPG          # 512 rows owned per rank
HL = H // TPG            # 4 heads per rank
DKL = HL * DK            # 256
EPS = 1e-5
GROUPS = [[0, 1, 2, 3], [4, 5, 6, 7]]
NT512 = T // 512         # 4
NTB = T // P             # 16
NFB = DFF // P           # 32
NCH = 4                  # local 128-row LayerNorm chunks


def round_fp32r(x: np.ndarray) -> np.ndarray:
    u = np.ascontiguousarray(x, dtype=np.float32).view(np.uint32)
    return ((u.astype(np.uint64) + 0x800) & 0xFFFFF000).astype(np.uint32).view(np.float32)


def to_bf16(x: np.ndarray) -> np.ndarray:
    return np.ascontiguousarray(np.asarray(x)).astype(mybir.dt.np(BF16))


def build_kernel(with_collectives=True, rep=1, stop_after=None):
    nc = bacc.Bacc("TRN2", target_bir_lowering=False, num_devices=NC)
    with tile.TileContext(nc) as tc, ExitStack() as top:
        dram = top.enter_context(tc.tile_pool(name="dram", bufs=1, space="DRAM"))

        def din(name, shape, dtype=F32R):
            return dram.tile(shape, dtype, kind="ExternalInput", uniquify=False, name=name)

        # ---------- I/O ----------
        xT = din("xT", [D, T], BF16)
        x_rows = din("x_rows", [TOWN, D], F32)      # strided-own rows + folds
        encT = din("encT", [D, T], BF16)
        saq_w = din("saq_w", [D, DKL], BF16); sak_w = din("sak_w", [D, DKL], BF16); sav_w = din("sav_w", [D, DKL], BF16)
        caq_w = din("caq_w", [D, D], BF16)
        cak_w = din("cak_w", [D, D], BF16); cav_w = din("cav_w", [D, D], BF16)
        qk_b = din("qk_b", [P, 2, 2], F32)          # [part, pair, (saq,sak)]
        cab = din("cab", [P, 8, 2], F32)            # [part, pair, (caq,cak)]
        sao_w = din("sao_w", [DKL, D], BF16); cao_w = din("cao_w", [D, D], BF16)
        w1 = din("w1", [D, DFF], BF16); b1 = din("b1", [P, NFB], F32)
        w2 = din("w2", [DFF, D], BF16)
        ln_g = din("ln_g", [3, 1, D], F32); ln_b = din("ln_b", [3, 1, D], F32)
        out = dram.tile([TOWN, D], F32, kind="ExternalOutput", uniquify=False, name="out")

        rs_in = [dram.tile([T, D], BF16, name="rs0_in")]
        rs_out = [dram.tile([TOWN, D], BF16, name="rs0_out")]

        # ---------- persistent SBUF ----------
        const = top.enter_context(tc.tile_pool(name="const", bufs=1))
        ident = const.tile([P, P], F32, name="ident")
        make_identity(nc, ident)
        eps_t = const.tile([P, 1], F32, name="eps_t")
        nc.vector.memset(eps_t[:], EPS)
        onesc_f = const.tile([P, NTB, H, 1], F32, name="onesc_f")
        nc.vector.memset(onesc_f[:], 1.0)

        qkb_sb = const.tile([P, 2, 2], F32, name="qkb_sb")
        nc.sync.dma_start(out=qkb_sb[:], in_=qk_b[:])
        cab_sb = const.tile([P, 8, 2], F32, name="cab_sb")
        nc.sync.dma_start(out=cab_sb[:], in_=cab[:])

        # causal diagonal masks: mask_j[k,q] = 1 if (q - 128*j - k) >= 0
        mask_sb = [const.tile([P, 512], BF16, name=f"mask_sb{j}") for j in range(4)]
        masks_f, free_masks_f = tc.tile([P, 4, 512], F32, name="masks_f")
        nc.gpsimd.memset(masks_f[:], 1.0)
        for j in range(4):
            nc.gpsimd.affine_select(out=masks_f[:, j, :], in_=masks_f[:, j, :],
                                    compare_op=ALU.is_ge, fill=0.0,
                                    base=-128 * j, pattern=[[1, 512]],
                                    channel_multiplier=-1)
        for j in range(4):
            nc.scalar.copy(mask_sb[j][:], masks_f[:, j, :])
        free_masks_f()

        # ================= helpers =================
        def project_qk(tag, dst, w_sb, bcol, rhs_fn, act_only=False):
            """dst [128,2,T]: per head pair out^T = W^T @ src^T, + bias."""
            with ExitStack() as hs:
                ps = hs.enter_context(tc.tile_pool(name=f"{tag}_psqk", bufs=4, space="PSUM"))
                for tck in range(NT512):
                    psts = [ps.tile([P, 512], F32, tag="proj", name=f"{tag}_pqk{bcol}_{p}_{tck}")
                            for p in range(2)]
                    for dblk in range(8):
                        rt = rhs_fn(dblk, tck)
                        for p in range(2):
                            nc.tensor.matmul(out=psts[p][:],
                                             lhsT=w_sb[:, dblk, p * 128:(p + 1) * 128],
                                             rhs=rt, start=(dblk == 0), stop=(dblk == 7))
                    for p in range(2):
                        if p == 0 or act_only:
                            nc.scalar.activation(out=dst[:, p, tck * 512:(tck + 1) * 512],
                                                 in_=psts[p][:], func=AF.Identity,
                                                 bias=qkb_sb[:, p, bcol:bcol + 1])
                        else:
                            nc.vector.tensor_scalar_add(
                                out=dst[:, p, tck * 512:(tck + 1) * 512],
                                in0=psts[p][:], scalar1=qkb_sb[:, p, bcol:bcol + 1])

        def project_qkv_shared(tag, QT, KT, Vp, wq_sb, wk_sb, wv_sb,
                               bq_col, bk_col, src_fn, act_only=False):
            """Q (optional), K, V projections sharing one streamed pass over the
            transposed source. src_fn(dblk, tck) -> AP [128,512] fp32r.
            act_only: put all PSUM evictions on the Act engine (keeps DVE free)."""
            with ExitStack() as hs:
                ps_qk = hs.enter_context(tc.tile_pool(name=f"{tag}_psqk", bufs=2, space="PSUM"))
                ps_v = hs.enter_context(tc.tile_pool(name=f"{tag}_psv", bufs=4, space="PSUM"))
                for tck in range(NT512):
                    psq = ([ps_qk.tile([P, 512], F32, tag="q", name=f"{tag}_psq{p}_{tck}")
                            for p in range(2)] if QT is not None else None)
                    psk = [ps_qk.tile([P, 512], F32, tag="k", name=f"{tag}_psk{p}_{tck}")
                           for p in range(2)]
                    psv = [ps_v.tile([P, DKL], F32, tag="v", name=f"{tag}_psv{j}_{tck}")
                           for j in range(4)]
                    for dblk in range(8):
                        xt = src_fn(dblk, tck)
                        first, last = (dblk == 0), (dblk == 7)
                        for p in range(2):
                            if psq is not None:
                                nc.tensor.matmul(out=psq[p][:],
                                                 lhsT=wq_sb[:, dblk, p * 128:(p + 1) * 128],
                                                 rhs=xt, start=first, stop=last,
                                                 skip_group_check=True)
                            nc.tensor.matmul(out=psk[p][:],
                                             lhsT=wk_sb[:, dblk, p * 128:(p + 1) * 128],
                                             rhs=xt, start=first, stop=last,
                                             skip_group_check=True)
                        for j in range(4):
                            nc.tensor.matmul(out=psv[j][:],
                                             lhsT=xt[:, j * 128:(j + 1) * 128],
                                             rhs=wv_sb[:, dblk, :],
                                             start=first, stop=last,
                                             skip_group_check=True)
                    for p in range(2):
                        if psq is not None:
                            if p == 0 or act_only:
                                nc.scalar.activation(out=QT[:, p, tck * 512:(tck + 1) * 512],
                                                     in_=psq[p][:], func=AF.Identity,
                                                     bias=qkb_sb[:, p, bq_col:bq_col + 1])
                            else:
                                nc.vector.tensor_scalar_add(
                                    out=QT[:, p, tck * 512:(tck + 1) * 512],
                                    in0=psq[p][:], scalar1=qkb_sb[:, p, bq_col:bq_col + 1])
                        if p == 0 or act_only:
                            nc.scalar.activation(out=KT[:, p, tck * 512:(tck + 1) * 512],
                                                 in_=psk[p][:], func=AF.Identity,
                                                 bias=qkb_sb[:, p, bk_col:bk_col + 1])
                        else:
                            nc.vector.tensor_scalar_add(
                                out=KT[:, p, tck * 512:(tck + 1) * 512],
                                in0=psk[p][:], scalar1=qkb_sb[:, p, bk_col:bk_col + 1])
                    for j in range(4):
                        tb = tck * 4 + j
                        vout = Vp[:, tb, :].rearrange("p (h v) -> p h v", v=65)[:, :, 0:64]
                        vin = psv[j][:].rearrange("p (h v) -> p h v", v=64)
                        if act_only or j % 2 == 0:
                            nc.scalar.copy(vout, vin)
                        else:
                            nc.vector.tensor_copy(vout, vin)
            nc.scalar.copy(
                Vp[:].rearrange("p t (h v) -> p t h v", v=65)[:, :, :, 64:65],
                onesc_f[:, :, 0:HL, :])

        def attention_block(tag, QT, KT, Vp, attnT, causal, loc, fillers=None):
            """Attention (head pairs x q-chunks); fillers: deque of closures
            emitting independent PE work, one popped per key-block to keep the
            tensor engine's busy-streak (pstate) alive across exp waits."""
            ps_sc = loc.enter_context(tc.tile_pool(name=f"{tag}_ps_sc", bufs=2, space="PSUM"))
            ps_av = loc.enter_context(tc.tile_pool(name=f"{tag}_ps_av", bufs=1, space="PSUM"))
            sb_pt = loc.enter_context(tc.tile_pool(name=f"{tag}_pt", bufs=4))
            sb_av = loc.enter_context(tc.tile_pool(name=f"{tag}_av", bufs=3))

            for j in range(NCH):
                nkb = (j + 1) * 4 if causal else NTB
                q_sl = slice(j * 512, (j + 1) * 512)
                for p in range(2):
                    avps = [ps_av.tile([65, 512], F32, tag=f"av{h}",
                                       name=f"{tag}_avps{p}_{j}_{h}")
                            for h in range(2)]

                    def emit_av(kb, pt):
                        first, last = (kb == 0), (kb == nkb - 1)
                        for h in range(2):
                            vcol = slice((2 * p + h) * 65, (2 * p + h + 1) * 65)
                            nc.tensor.matmul(out=avps[h][:],
                                             lhsT=Vp[:, kb, vcol],
                                             rhs=pt[:, h, :], start=first,
                                             stop=last, skip_group_check=True)

                    pending = None
                    for kb in range(nkb):
                        k_sl = slice(kb * 128, (kb + 1) * 128)
                        psS = ps_sc.tile([P, 2, 512], F32, tag="sc",
                                         name=f"{tag}_sc{p}_{j}_{kb}")
                        nc.tensor.matmul(out=psS[:, 0, :], lhsT=KT[0:64, p, k_sl],
                                         rhs=QT[0:64, p, q_sl], start=True, stop=True)
                        nc.tensor.matmul(out=psS[:, 1, :], lhsT=KT[64:128, p, k_sl],
                                         rhs=QT[64:128, p, q_sl], start=True, stop=True)
                        pt = sb_pt.tile([P, 2, 512], BF16, tag="pt",
                                        name=f"{tag}_pt{p}_{j}_{kb}")
                        nc.scalar.activation(out=pt[:], in_=psS[:], func=AF.Exp,
                                             scale=0.125)
                        if causal and kb >= j * 4:
                            mj = mask_sb[kb - j * 4]
                            nc.gpsimd.tensor_mul(pt[:, 0, :], pt[:, 0, :], mj[:])
                            nc.gpsimd.tensor_mul(pt[:, 1, :], pt[:, 1, :], mj[:])
                        if fillers and (kb % 5 == 2 or (causal and kb >= j * 4 + 2)):
                            fillers.popleft()()
                        if pending is not None:
                            emit_av(*pending)
                        pending = (kb, pt)
                    emit_av(*pending)
                    if fillers:
                        fillers.popleft()()
                    for h in range(2):
                        den = sb_av.tile([1, 512], F32, tag="den0",
                                         name=f"{tag}_den0_{p}_{j}_{h}")
                        nc.vector.reciprocal(den[:], avps[h][64:65, :])
                        bc = sb_av.tile([64, 512], F32, tag="bc",
                                        name=f"{tag}_bc_{p}_{j}_{h}")
                        nc.gpsimd.partition_broadcast(bc[:], den[:], channels=64)
                        if h == 0:
                            nc.vector.tensor_mul(attnT[0:64, p, q_sl],
                                                 avps[h][0:64, :], bc[:])
                        else:
                            nc.vector.tensor_mul(attnT[64:128, p, q_sl],
                                                 avps[h][0:64, :], bc[:])

        def wo_rs(tag, attnT, wo_sb, rs_in_t, rs_out_t, loc):
            ps_y = loc.enter_context(tc.tile_pool(name=f"{tag}_psy", bufs=2, space="PSUM"))
            sb_y = loc.enter_context(tc.tile_pool(name=f"{tag}_ysb", bufs=4))
            for tb in range(NTB):
                col = tb * 128
                ysb = sb_y.tile([P, 2, 512], BF16, tag="ysb", name=f"{tag}_ysb_{tb}")
                for s in range(2):
                    psY = ps_y.tile([P, 512], F32, tag="y", name=f"{tag}_psY_{tb}_{s}")
                    for p in range(2):
                        nc.tensor.matmul(out=psY[:],
                                         lhsT=attnT[:, p, col:col + 128],
                                         rhs=wo_sb[:, p, s * 512:(s + 1) * 512],
                                         start=(p == 0), stop=(p == 1))
                    if tb % 2 == 0:
                        nc.scalar.copy(ysb[:, s, :], psY[:])
                    else:
                        nc.vector.tensor_copy(ysb[:, s, :], psY[:])
                nc.sync.dma_start(out=rs_in_t[col:col + 128, :],
                                  in_=ysb[:].rearrange("p s f -> p (s f)"))
            if with_collectives:
                nc.gpsimd.collective_compute(
                    "ReduceScatter", ALU.add, replica_groups=GROUPS,
                    ins=[rs_in_t[:]], outs=[rs_out_t[:]])

        def ln_vec(tag, j, rs_out_t, res_ap, lni, a_dst, sb_ln, gt, bt):
            """LayerNorm (vector part) of local 128-row chunk j."""
            ybf = sb_ln.tile([P, D], BF16, tag="ybf", name=f"{tag}_ybf_{j}")
            nc.sync.dma_start(out=ybf[:], in_=rs_out_t[j * P:(j + 1) * P, :])
            yown = sb_ln.tile([P, D], F32, tag="yown", name=f"{tag}_yown_{j}")
            nc.vector.tensor_add(yown[:], ybf[:], res_ap)
            ln_norm(tag, j, yown, a_dst, sb_ln, gt, bt)

        def ln_norm(tag, j, yown, a_dst, sb_ln, gt, bt):
            st = sb_ln.tile([P, 2, 6], F32, tag="st", name=f"{tag}_st_{j}")
            nc.vector.bn_stats(out=st[:, 0, :], in_=yown[:, 0:512])
            nc.vector.bn_stats(out=st[:, 1, :], in_=yown[:, 512:1024])
            mv = sb_ln.tile([P, 2], F32, tag="mv", name=f"{tag}_mv_{j}")
            nc.vector.bn_aggr(out=mv[:], in_=st[:])
            nc.scalar.activation(out=mv[:, 1:2], in_=mv[:, 1:2], func=AF.Sqrt,
                                 bias=eps_t[:])
            nc.vector.reciprocal(mv[:, 1:2], mv[:, 1:2])
            nc.vector.tensor_scalar(out=a_dst[:, j, :], in0=yown[:],
                                    scalar1=mv[:, 0:1], scalar2=mv[:, 1:2],
                                    op0=ALU.subtract, op1=ALU.mult)
            nc.vector.tensor_mul(a_dst[:, j, :], a_dst[:, j, :], gt[:])
            nc.vector.tensor_add(a_dst[:, j, :], a_dst[:, j, :], bt[:])

        def ln_transpose(tag, j, a_dst, at_dst, ps_tr):
            for dblk in range(8):
                pst = ps_tr.tile([P, P], F32, tag="tr", name=f"{tag}_tr_{j}_{dblk}")
                nc.tensor.transpose(pst[:], a_dst[:, j, dblk * 128:(dblk + 1) * 128],
                                    ident[:])
                nc.vector.tensor_copy(at_dst[:, dblk, j * 128:(j + 1) * 128], pst[:])

        # ================= phases =================
        resid = top.enter_context(tc.tile_pool(name="resid", bufs=1))
        a1_sb = resid.tile([P, 4, D], F32, name="a1_sb")
        a2_sb = resid.tile([P, 4, D], F32, name="a2_sb")
        a2t_sb = resid.tile([P, 8, TOWN], BF16, name="a2t_sb")
        lng_pool = top.enter_context(tc.tile_pool(name="lng", bufs=1))

        def emit_body(stop=None):
            with ExitStack() as ph:
                # V2 (cross attention values) is filled DURING self-attention
                # as PE filler work, so its pool spans both blocks.
                v2_pool = ph.enter_context(tc.tile_pool(name="D_v2", bufs=1))
                att2_pool = ph.enter_context(tc.tile_pool(name="D_att", bufs=1))
                V2 = v2_pool.tile([P, NTB, H * 65], BF16, name="D_V")
                attnT2 = att2_pool.tile([P, 8, TOWN], BF16, name="D_attnT")
                # --- self attention block (+ V2-projection filler) ---
                with ExitStack() as phs:
                    qkv = phs.enter_context(tc.tile_pool(name="A_qkv", bufs=1))
                    att_pool = phs.enter_context(tc.tile_pool(name="A_att", bufs=1))
                    QT1 = qkv.tile([P, 2, T], BF16, name="A_QT")
                    KT1 = qkv.tile([P, 2, T], BF16, name="A_KT")
                    V1 = qkv.tile([P, NTB, HL * 65], BF16, name="A_V")
                    attnT1 = att_pool.tile([P, 2, T], BF16, name="A_attnT")
                    with ExitStack() as loc:
                        sbw = loc.enter_context(tc.tile_pool(name="A_w", bufs=1))
                        xs_pool = loc.enter_context(tc.tile_pool(name="A_xs", bufs=8))
                        wq_sb = sbw.tile([P, 8, DKL], BF16, name="A_wq")
                        nc.sync.dma_start(out=wq_sb[:], in_=saq_w[:].rearrange("(n p) m -> p n m", p=P))
                        wk_sb = sbw.tile([P, 8, DKL], BF16, name="A_wk")
                        nc.sync.dma_start(out=wk_sb[:], in_=sak_w[:].rearrange("(n p) m -> p n m", p=P))
                        wv_sb = sbw.tile([P, 8, DKL], BF16, name="A_wv")
                        nc.sync.dma_start(out=wv_sb[:], in_=sav_w[:].rearrange("(n p) m -> p n m", p=P))

                        def x_src(dblk, tck):
                            t = xs_pool.tile([P, 512], BF16, tag="xs", name=f"A_xs_{dblk}_{tck}")
                            nc.sync.dma_start(out=t[:], in_=xT[dblk * 128:(dblk + 1) * 128,
                                                              tck * 512:(tck + 1) * 512])
                            return t[:]

                        project_qkv_shared("A", QT1, KT1, V1, wq_sb, wk_sb, wv_sb, 0, 1, x_src)
                    if stop == "qkv1":
                        return
                    with ExitStack() as loc:
                        # V2 projection filler units (enc @ Wv, all 16 heads)
                        sbwv = loc.enter_context(tc.tile_pool(name="D_wv_p", bufs=1))
                        enc_pool = loc.enter_context(tc.tile_pool(name="D_encV", bufs=2))
                        ps_v = loc.enter_context(tc.tile_pool(name="D_psv", bufs=1, space="PSUM"))
                        wv2_sb = sbwv.tile([P, 8, D], BF16, name="D_wv")
                        nc.sync.dma_start(out=wv2_sb[:], in_=cav_w[:].rearrange("(n p) m -> p n m", p=P))
                        enc_tiles = {}

                        def v_unit(tck, j):
                            def emit():
                                if tck not in enc_tiles:
                                    t = enc_pool.tile([P, 8, 512], BF16, tag="enc",
                                                      name=f"D_encV_{tck}")
                                    nc.sync.dma_start(
                                        out=t[:],
                                        in_=encT[:, tck * 512:(tck + 1) * 512]
                                        .rearrange("(n p) m -> p n m", p=P))
                                    enc_tiles[tck] = t
                                    if tck - 2 in enc_tiles:
                                        del enc_tiles[tck - 2]
                                enct = enc_tiles[tck]
                                tb = tck * 4 + j
                                psv = ps_v.tile([P, 2, 512], F32, tag="v",
                                                name=f"D_psv_{tck}_{j}")
                                for dblk in range(8):
                                    for sh in range(2):
                                        nc.tensor.matmul(
                                            out=psv[:, sh, :],
                                            lhsT=enct[:, dblk, j * 128:(j + 1) * 128],
                                            rhs=wv2_sb[:, dblk, sh * 512:(sh + 1) * 512],
                                            start=(dblk == 0), stop=(dblk == 7),
                                            skip_group_check=True)
                                vout = V2[:, tb, :].rearrange("p (h v) -> p h v", v=65)[:, :, 0:64]
                                vin = psv[:].rearrange("p s (g v) -> p (s g) v", v=64)
                                nc.vector.tensor_copy(vout, vin)
                            return emit

                        from collections import deque
                        fillers = deque(v_unit(tck, j)
                                        for tck in range(NT512) for j in range(4))
                        with ExitStack() as loc2:
                            attention_block("B", QT1, KT1, V1, attnT1, True, loc2,
                                            fillers=fillers)
                        while fillers:
                            fillers.popleft()()
                        nc.scalar.copy(
                            V2[:].rearrange("p t (h v) -> p t h v", v=65)[:, :, :, 64:65],
                            onesc_f[:])
                    if stop == "att1":
                        return
                    # Wo + single ReduceScatter (hidden under the K-pass below)
                    with ExitStack() as loc:
                        sbwo = loc.enter_context(tc.tile_pool(name="B_wo", bufs=1))
                        wo_sb = sbwo.tile([P, 2, D], BF16, name="B_wo_sb")
                        nc.sync.dma_start(out=wo_sb[:], in_=sao_w[:].rearrange("(n p) m -> p n m", p=P))
                        wo_rs("C", attnT1, wo_sb, rs_in[0], rs_out[0], loc)
                    if stop == "wo1":
                        return
                # --- cross attention (q-sharded: all 16 heads, own rows) ---
                qkv2 = ph.enter_context(tc.tile_pool(name="D_qkv", bufs=1))
                KT2 = qkv2.tile([P, 8, T], BF16, name="D_KT")
                QT2 = qkv2.tile([P, 8, TOWN], BF16, name="D_QT")
                a1t_pool = ph.enter_context(tc.tile_pool(name="a1t_pool", bufs=1))
                a1t_sb = a1t_pool.tile([P, 8, TOWN], BF16, name="a1t_sb")
                # K-pass (full heads) — overlaps the RS1 collective
                with ExitStack() as loc:
                    sbw = loc.enter_context(tc.tile_pool(name="D_wk_p", bufs=1))
                    enc_pool = loc.enter_context(tc.tile_pool(name="D_encK", bufs=2))
                    ps_k = loc.enter_context(tc.tile_pool(name="D_psk", bufs=2, space="PSUM"))
                    wk_sb = sbw.tile([P, 8, D], BF16, name="D_wk")
                    nc.sync.dma_start(out=wk_sb[:], in_=cak_w[:].rearrange("(n p) m -> p n m", p=P))
                    for tck in range(NT512):
                        enct = enc_pool.tile([P, 8, 512], BF16, tag="enc",
                                             name=f"D_encK_{tck}")
                        nc.sync.dma_start(out=enct[:],
                                          in_=encT[:, tck * 512:(tck + 1) * 512]
                                          .rearrange("(n p) m -> p n m", p=P))
                        for p in range(8):
                            psk = ps_k.tile([P, 512], F32, tag="k", name=f"D_psk_{tck}_{p}")
                            for dblk in range(8):
                                nc.tensor.matmul(out=psk[:],
                                                 lhsT=wk_sb[:, dblk, p * 128:(p + 1) * 128],
                                                 rhs=enct[:, dblk, :],
                                                 start=(dblk == 0), stop=(dblk == 7))
                            nc.scalar.activation(out=KT2[:, p, tck * 512:(tck + 1) * 512],
                                                 in_=psk[:], func=AF.Identity,
                                                 bias=cab_sb[:, p, 1:2])
                if stop == "kv2":
                    return
                # LN1 (+ residual) on own rows; transpose a1 -> a1t (bf16)
                with ExitStack() as ph2:
                    xr_pool = ph2.enter_context(tc.tile_pool(name="xr_pool", bufs=1))
                    sb_ln = ph2.enter_context(tc.tile_pool(name="C2_ln", bufs=2))
                    ps_tr = ph2.enter_context(tc.tile_pool(name="C2_ps_tr", bufs=4, space="PSUM"))
                    x_rows_sb = xr_pool.tile([P, 4, D], F32, name="x_rows_sb")
                    nc.sync.dma_start(out=x_rows_sb[:],
                                      in_=x_rows[:].rearrange("(n p) m -> p n m", p=P))
                    g1 = lng_pool.tile([P, D], F32, name="g1")
                    nc.sync.dma_start(out=g1[:], in_=ln_g[0].to_broadcast((P, D)))
                    bt1 = lng_pool.tile([P, D], F32, name="bt1")
                    nc.sync.dma_start(out=bt1[:], in_=ln_b[0].to_broadcast((P, D)))
                    for j in range(NCH):
                        ln_vec("C2", j, rs_out[0], x_rows_sb[:, j, :], 0, a1_sb,
                               sb_ln, g1, bt1)
                    for j in range(NCH):
                        ln_transpose("C2", j, a1_sb, a1t_sb, ps_tr)
                if stop == "ln1":
                    return
                # Q (all heads) from own a1^T — no collective needed
                with ExitStack() as loc:
                    sbw = loc.enter_context(tc.tile_pool(name="D_wq_p", bufs=1))
                    ps_q = loc.enter_context(tc.tile_pool(name="D_psq", bufs=2, space="PSUM"))
                    wq_sb = sbw.tile([P, 8, D], BF16, name="D_wq")
                    nc.sync.dma_start(out=wq_sb[:], in_=caq_w[:].rearrange("(n p) m -> p n m", p=P))
                    for p in range(8):
                        psq = ps_q.tile([P, 512], F32, tag="q", name=f"D_psq_{p}")
                        for dblk in range(8):
                            nc.tensor.matmul(out=psq[:],
                                             lhsT=wq_sb[:, dblk, p * 128:(p + 1) * 128],
                                             rhs=a1t_sb[:, dblk, :],
                                             start=(dblk == 0), stop=(dblk == 7))
                        nc.scalar.activation(out=QT2[:, p, :], in_=psq[:],
                                             func=AF.Identity, bias=cab_sb[:, p, 0:1])
                if stop == "qkv2":
                    return
                # q-sharded cross attention over all 16 heads
                with ExitStack() as loc:
                    ps_sc = loc.enter_context(tc.tile_pool(name="E_ps_sc", bufs=2, space="PSUM"))
                    ps_av = loc.enter_context(tc.tile_pool(name="E_ps_av", bufs=2, space="PSUM"))
                    sb_pt = loc.enter_context(tc.tile_pool(name="E_pt", bufs=4))
                    sb_av = loc.enter_context(tc.tile_pool(name="E_av", bufs=3))
                    for p in range(8):
                        avps = [ps_av.tile([65, 512], F32, tag=f"av{h}",
                                           name=f"E_avps{p}_{h}") for h in range(2)]

                        def emit_av(kb, pt):
                            first, last = (kb == 0), (kb == NTB - 1)
                            for h in range(2):
                                vcol = slice((2 * p + h) * 65, (2 * p + h + 1) * 65)
                                nc.tensor.matmul(out=avps[h][:],
                                                 lhsT=V2[:, kb, vcol],
                                                 rhs=pt[:, h, :], start=first,
                                                 stop=last, skip_group_check=True)

                        pending = None
                        for kb in range(NTB):
                            k_sl = slice(kb * 128, (kb + 1) * 128)
                            psS = ps_sc.tile([P, 2, 512], F32, tag="sc",
                                             name=f"E_sc{p}_{kb}")
                            nc.tensor.matmul(out=psS[:, 0, :], lhsT=KT2[0:64, p, k_sl],
                                             rhs=QT2[0:64, p, :], start=True, stop=True)
                            nc.tensor.matmul(out=psS[:, 1, :], lhsT=KT2[64:128, p, k_sl],
                                             rhs=QT2[64:128, p, :], start=True, stop=True)
                            pt = sb_pt.tile([P, 2, 512], BF16, tag="pt",
                                            name=f"E_pt{p}_{kb}")
                            nc.scalar.activation(out=pt[:], in_=psS[:], func=AF.Exp,
                                                 scale=0.125)
                            if pending is not None:
                                emit_av(*pending)
                            pending = (kb, pt)
                        emit_av(*pending)
                        for h in range(2):
                            den = sb_av.tile([1, 512], F32, tag="den0",
                                             name=f"E_den0_{p}_{h}")
                            nc.vector.reciprocal(den[:], avps[h][64:65, :])
                            bc = sb_av.tile([64, 512], F32, tag="bc",
                                            name=f"E_bc_{p}_{h}")
                            nc.gpsimd.partition_broadcast(bc[:], den[:], channels=64)
                            if h == 0:
                                nc.vector.tensor_mul(attnT2[0:64, p, :],
                                                     avps[h][0:64, :], bc[:])
                            else:
                                nc.vector.tensor_mul(attnT2[64:128, p, :],
                                                     avps[h][0:64, :], bc[:])
                if stop == "att2":
                    return
                # Wo (fully local contraction over all 16 heads) + LN2 fused
                with ExitStack() as ph2:
                    sbwo = ph2.enter_context(tc.tile_pool(name="F_wo", bufs=1))
                    sb_ln = ph2.enter_context(tc.tile_pool(name="F2_ln", bufs=2))
                    ps_y = ph2.enter_context(tc.tile_pool(name="F_psy", bufs=2, space="PSUM"))
                    ps_tr = ph2.enter_context(tc.tile_pool(name="F2_ps_tr", bufs=4, space="PSUM"))
                    wo_sb = sbwo.tile([P, 8, D], BF16, name="F_wo_sb")
                    nc.sync.dma_start(out=wo_sb[:], in_=cao_w[:].rearrange("(n p) m -> p n m", p=P))
                    g2 = lng_pool.tile([P, D], F32, name="g2")
                    nc.sync.dma_start(out=g2[:], in_=ln_g[1].to_broadcast((P, D)))
                    bt2 = lng_pool.tile([P, D], F32, name="bt2")
                    nc.sync.dma_start(out=bt2[:], in_=ln_b[1].to_broadcast((P, D)))
                    for tb in range(4):
                        yown = sb_ln.tile([P, D], F32, tag="yo2", name=f"F_yo2_{tb}")
                        for s in range(2):
                            psY = ps_y.tile([P, 512], F32, tag="y", name=f"F_psY_{tb}_{s}")
                            for p in range(8):
                                nc.tensor.matmul(out=psY[:],
                                                 lhsT=attnT2[:, p, tb * 128:(tb + 1) * 128],
                                                 rhs=wo_sb[:, p, s * 512:(s + 1) * 512],
                                                 start=(p == 0), stop=(p == 7))
                            if s == 0:
                                nc.scalar.copy(yown[:, 0:512], psY[:])
                            else:
                                nc.vector.tensor_copy(yown[:, 512:1024], psY[:])
                        nc.vector.tensor_add(yown[:], yown[:], a1_sb[:, tb, :])
                        ln_norm("F2", tb, yown, a2_sb, sb_ln, g2, bt2)
                    for j in range(NCH):
                        ln_transpose("F2", j, a2_sb, a2t_sb, ps_tr)
            if stop == "ln2":
                return

            # --- FFN (row-sharded) ---
            with ExitStack() as ph:
                hpool = ph.enter_context(tc.tile_pool(name="G_h", bufs=1))
                sb_ln = ph.enter_context(tc.tile_pool(name="H_ln", bufs=3))
                b1_sb = hpool.tile([P, NFB], F32, name="b1_sb")
                nc.sync.dma_start(out=b1_sb[:], in_=b1[:])
                g3 = hpool.tile([P, D], F32, name="g3")
                nc.sync.dma_start(out=g3[:], in_=ln_g[2].to_broadcast((P, D)))
                b3 = hpool.tile([P, D], F32, name="b3")
                nc.sync.dma_start(out=b3[:], in_=ln_b[2].to_broadcast((P, D)))
                hT_sb = hpool.tile([P, NFB, TOWN], BF16, name="hT_sb")
                y2_sb = hpool.tile([P, 4, D], F32, name="y2_sb")
                with ExitStack() as loc:
                    wpool = loc.enter_context(tc.tile_pool(name="G_w", bufs=2))
                    ps = loc.enter_context(tc.tile_pool(name="G_ps", bufs=3, space="PSUM"))
                    for fq in range(NFB // 4):
                        w1t = wpool.tile([P, 8, 512], BF16, tag="w1", name=f"G_w1_{fq}")
                        nc.sync.dma_start(out=w1t[:],
                                          in_=w1[:, fq * 512:(fq + 1) * 512]
                                          .rearrange("(n p) m -> p n m", p=P))
                        for fs in range(4):
                            fb = fq * 4 + fs
                            psH = ps.tile([P, 512], F32, tag="h", name=f"G_psH_{fb}")
                            for dblk in range(8):
                                nc.tensor.matmul(out=psH[:],
                                                 lhsT=w1t[:, dblk, fs * 128:(fs + 1) * 128],
                                                 rhs=a2t_sb[:, dblk, :],
                                                 start=(dblk == 0), stop=(dblk == 7))
                            nc.scalar.activation(out=hT_sb[:, fb, :], in_=psH[:], func=AF.Relu,
                                                 bias=b1_sb[:, fb:fb + 1])
                if stop == "ffn1":
                    return
                with ExitStack() as loc:
                    wpool = loc.enter_context(tc.tile_pool(name="H_w", bufs=8))
                    ps_y2 = loc.enter_context(tc.tile_pool(name="H_ps", bufs=1, space="PSUM"))
                    for s in range(2):
                        psY2 = [ps_y2.tile([P, 512], F32, tag=f"y2_{tb}",
                                           name=f"H_psY2_{s}_{tb}") for tb in range(4)]
                        for fb in range(NFB):
                            w2t = wpool.tile([P, 512], BF16, tag="w2", name=f"H_w2_{s}_{fb}")
                            nc.sync.dma_start(out=w2t[:], in_=w2[fb * 128:(fb + 1) * 128,
                                                               s * 512:(s + 1) * 512])
                            for tb in range(4):
                                nc.tensor.matmul(out=psY2[tb][:],
                                                 lhsT=hT_sb[:, fb, tb * 128:(tb + 1) * 128],
                                                 rhs=w2t[:], start=(fb == 0),
                                                 stop=(fb == NFB - 1), skip_group_check=True)
                        for tb in range(4):
                            if tb % 2 == 0:
                                nc.scalar.copy(y2_sb[:, tb, s * 512:(s + 1) * 512], psY2[tb][:])
                            else:
                                nc.vector.tensor_copy(y2_sb[:, tb, s * 512:(s + 1) * 512],
                                                      psY2[tb][:])
                for tb in range(4):
                    nc.vector.tensor_add(y2_sb[:, tb, :], y2_sb[:, tb, :], a2_sb[:, tb, :])
                    st = sb_ln.tile([P, 2, 6], F32, tag="st", name=f"H_st_{tb}")
                    nc.vector.bn_stats(out=st[:, 0, :], in_=y2_sb[:, tb, 0:512])
                    nc.vector.bn_stats(out=st[:, 1, :], in_=y2_sb[:, tb, 512:1024])
                    mv = sb_ln.tile([P, 2], F32, tag="mv", name=f"H_mv_{tb}")
                    nc.vector.bn_aggr(out=mv[:], in_=st[:])
                    nc.scalar.activation(out=mv[:, 1:2], in_=mv[:, 1:2], func=AF.Sqrt,
                                         bias=eps_t[:])
                    nc.vector.reciprocal(mv[:, 1:2], mv[:, 1:2])
                    osb = sb_ln.tile([P, D], F32, tag="osb", name=f"H_osb_{tb}")
                    nc.vector.tensor_scalar(out=osb[:], in0=y2_sb[:, tb, :],
                                            scalar1=mv[:, 0:1], scalar2=mv[:, 1:2],
                                            op0=ALU.subtract, op1=ALU.mult)
                    nc.vector.tensor_mul(osb[:], osb[:], g3[:])
                    nc.vector.tensor_add(osb[:], osb[:], b3[:])
                    nc.sync.dma_start(out=out[tb * 128:(tb + 1) * 128, :], in_=osb[:])

        for _rep in range(rep):
            emit_body(stop_after)

    nc.compile()
    return nc


_NC_CACHE = None


def _get_nc():
    global _NC_CACHE
    if _NC_CACHE is None:
        _NC_CACHE = build_kernel()
    return _NC_CACHE


def own_rows(c):
    """True row indices owned by rank c (within its batch), in local order."""
    return TOWN * c + np.arange(TOWN)


def make_in_maps(inputs):
    """Build the 8 per-core input dicts from the full problem inputs."""
    g = {k: np.asarray(v) for k, v in inputs.items()}
    la = g["lookahead_mask"]
    pm = g["padding_mask"]
    assert np.array_equal(la[0, 0], np.tril(np.ones((T, T), la.dtype))), \
        "kernel specialized for causal lookahead_mask"
    assert pm.min() == 1, "kernel specialized for all-ones padding_mask"

    r32 = round_fp32r
    # host-side bias folds (all fp64 for accuracy)
    sa_fold = (g["sa_bv"].astype(np.float64) @ g["sa_Wo"].astype(np.float64)
               + g["sa_bo"].astype(np.float64))                  # [D]
    cb = (g["ca_bv"].astype(np.float64) @ g["ca_Wo"].astype(np.float64)
          + g["ca_bo"].astype(np.float64))                       # [D]
    ln1_b = g["ln1_b"].astype(np.float64) + cb
    ln2_b = g["ln2_b"].astype(np.float64) + g["ff_b2"].astype(np.float64)
    ca_bq = g["ca_bq"].astype(np.float64) - cb @ g["ca_Wq"].astype(np.float64)
    b1_full = (g["ff_b1"].astype(np.float64)
               - g["ff_b2"].astype(np.float64) @ g["ff_W1"].astype(np.float64))

    qk_b_h = np.zeros((NC // 2, P, 2, 2), np.float32)
    for c in range(NC // 2):
        hsl = slice(DKL * c, DKL * (c + 1))
        for i, bias in enumerate((g["sa_bq"], g["sa_bk"])):
            qk_b_h[c, :, :, i] = np.asarray(bias)[hsl].reshape(2, 128).T.astype(np.float32)
    cab = np.zeros((P, 8, 2), np.float32)
    cab[:, :, 0] = ca_bq.reshape(8, 128).T.astype(np.float32)
    cab[:, :, 1] = np.asarray(g["ca_bk"]).reshape(8, 128).T.astype(np.float32)

    in_maps = []
    for r in range(NC):
        b, c = r // TPG, r % TPG
        hsl = slice(DKL * c, DKL * (c + 1))
        rows = own_rows(c)
        m = dict(
            xT=to_bf16(np.ascontiguousarray(g["x"][b].T)),
            x_rows=np.ascontiguousarray(
                g["x"][b][rows].astype(np.float64) + sa_fold).astype(np.float32),
            encT=to_bf16(np.ascontiguousarray(g["encoder_output"][b].T)),
            saq_w=to_bf16(g["sa_Wq"][:, hsl]), sak_w=to_bf16(g["sa_Wk"][:, hsl]),
            sav_w=to_bf16(g["sa_Wv"][:, hsl]),
            caq_w=to_bf16(g["ca_Wq"]),
            cak_w=to_bf16(g["ca_Wk"]), cav_w=to_bf16(g["ca_Wv"]),
            qk_b=qk_b_h[c], cab=cab,
            sao_w=to_bf16(g["sa_Wo"][hsl, :]), cao_w=to_bf16(g["ca_Wo"]),
            w1=to_bf16(g["ff_W1"]),
            b1=np.ascontiguousarray(
                b1_full.astype(np.float32).reshape(NFB, P).T),
            w2=to_bf16(g["ff_W2"]),
            ln_g=np.stack([g["ln1_g"], g["ln2_g"], g["ln3_g"]])[:, None].astype(np.float32),
            ln_b=np.stack([ln1_b.astype(np.float32), ln2_b.astype(np.float32),
                           g["ln3_b"]])[:, None].astype(np.float32),
        )
        in_maps.append(m)
    return in_maps


def assemble(results):
    outp = np.empty((B, T, D), np.float32)
    for r in range(NC):
        b, c = r // TPG, r % TPG
        outp[b][own_rows(c)] = results[r]["out"]
    return outp


def kernel(**inputs) -> np.ndarray:
    nc = _get_nc()
    in_maps = make_in_maps(inputs)
    res = run_bass_kernel_spmd(nc, in_maps, core_ids=list(range(NC)), trace=False)
    return assemble(res.results)


# revision 27
# speedup vs baseline: 1.0087x; 1.0087x over previous
"""Trainium2 Bass kernel for nn_DecoderLayer (B=2,T=2048,D=1024,H=16,dk=dv=64,dff=4096).

Sharding: 8 cores = 2 batch groups (data parallel) x 4 ranks; rank c owns
contiguous rows [512c, 512c+512) of its batch.
  - Self-attention: head-parallel (4 heads/core); S^T=[k,q] blocks; softmax
    denominator fused into AV as a ones-column of V; single AV matmul with
    full 128-key contraction. The cross-attention V projection (enc @ Wv,
    all 16 heads) is interleaved into self-attention as tensor-engine
    filler so the PE busy-streak (pstate) never drops.
  - Wo1 partials -> ONE bf16 ReduceScatter(add), hidden under the full-head
    cross K projection.
  - Cross-attention is Q-SHARDED: every rank computes all 16 heads for its
    own 512 rows, so Q2 needs only local LN1 output (no AllGather) and the
    Wo2 contraction is fully local (no second ReduceScatter).
  - FFN row-sharded, no collective. LayerNorms on own rows.
All biases are folded host-side: v-bias/Wo-bias via softmax sum-to-1 into
the LN residuals (x_rows, ln1_b, ln2_b) with Q-bias/FFN-b1 compensations.
Nearly all matmuls run in bf16 (error budget 2e-2, achieved ~1.6e-3);
residual/LN arithmetic stays fp32.
"""
from contextlib import ExitStack

import numpy as np

import concourse.bacc as bacc
import concourse.tile as tile
import concourse.mybir as mybir
from concourse.bass_utils import run_bass_kernel_spmd
from concourse.masks import make_identity

F32 = mybir.dt.float32
F32R = mybir.dt.float32r
BF16 = mybir.dt.bfloat16
AF = mybir.ActivationFunctionType
ALU = mybir.AluOpType
P = 128

B, T, D, H, DK, DV, DFF = 2, 2048, 1024, 16, 64, 64, 4096
NC, TPG = 8, 4
TOWN = T // TPG          # 512 rows owned per rank
HL = H // TPG            # 4 heads per rank
DKL = HL * DK            # 256
EPS = 1e-5
GROUPS = [[0, 1, 2, 3], [4, 5, 6, 7]]
NT512 = T // 512         # 4
NTB = T // P             # 16
NFB = DFF // P           # 32
NCH = 4                  # local 128-row LayerNorm chunks


def round_fp32r(x: np.ndarray) -> np.ndarray:
    u = np.ascontiguousarray(x, dtype=np.float32).view(np.uint32)
    return ((u.astype(np.uint64) + 0x800) & 0xFFFFF000).astype(np.uint32).view(np.float32)


def to_bf16(x: np.ndarray) -> np.ndarray:
    return np.ascontiguousarray(np.asarray(x)).astype(mybir.dt.np(BF16))


def build_kernel(with_collectives=True, rep=1, stop_after=None):
    nc = bacc.Bacc("TRN2", target_bir_lowering=False, num_devices=NC)
    with tile.TileContext(nc) as tc, ExitStack() as top:
        dram = top.enter_context(tc.tile_pool(name="dram", bufs=1, space="DRAM"))

        def din(name, shape, dtype=F32R):
            return dram.tile(shape, dtype, kind="ExternalInput", uniquify=False, name=name)

        # ---------- I/O ----------
        xT = din("xT", [D, T], BF16)
        x_rows = din("x_rows", [TOWN, D], F32)      # strided-own rows + folds
        encT = din("encT", [D, T], BF16)
        saq_w = din("saq_w", [D, DKL], BF16); sak_w = din("sak_w", [D, DKL], BF16); sav_w = din("sav_w", [D, DKL], BF16)
        caq_w = din("caq_w", [D, D], BF16)
        cak_w = din("cak_w", [D, D], BF16); cav_w = din("cav_w", [D, D], BF16)
        qk_b = din("qk_b", [P, 2, 2], F32)          # [part, pair, (saq,sak)]
        cab = din("cab", [P, 8, 2], F32)            # [part, pair, (caq,cak)]
        sao_w = din("sao_w", [DKL, D], BF16); cao_w = din("cao_w", [D, D], BF16)
        w1 = din("w1", [D, DFF], BF16); b1 = din("b1", [P, NFB], F32)
        w2 = din("w2", [DFF, D], BF16)
        ln_g = din("ln_g", [3, 1, D], F32); ln_b = din("ln_b", [3, 1, D], F32)
        out = dram.tile([TOWN, D], F32, kind="ExternalOutput", uniquify=False, name="out")

        rs_in = [dram.tile([T, D], BF16, name="rs0_in")]
        rs_out = [dram.tile([TOWN, D], BF16, name="rs0_out")]

        # ---------- persistent SBUF ----------
        const = top.enter_context(tc.tile_pool(name="const", bufs=1))
        ident = const.tile([P, P], F32, name="ident")
        make_identity(nc, ident)
        eps_t = const.tile([P, 1], F32, name="eps_t")
        nc.vector.memset(eps_t[:], EPS)
        onesc_f = const.tile([P, NTB, H, 1], F32, name="onesc_f")
        nc.vector.memset(onesc_f[:], 1.0)

        qkb_sb = const.tile([P, 2, 2], F32, name="qkb_sb")
        nc.sync.dma_start(out=qkb_sb[:], in_=qk_b[:])
        cab_sb = const.tile([P, 8, 2], F32, name="cab_sb")
        nc.sync.dma_start(out=cab_sb[:], in_=cab[:])

        # causal diagonal masks: mask_j[k,q] = 1 if (q - 128*j - k) >= 0
        mask_sb = [const.tile([P, 512], BF16, name=f"mask_sb{j}") for j in range(4)]
        masks_f, free_masks_f = tc.tile([P, 4, 512], F32, name="masks_f")
        nc.gpsimd.memset(masks_f[:], 1.0)
        for j in range(4):
            nc.gpsimd.affine_select(out=masks_f[:, j, :], in_=masks_f[:, j, :],
                                    compare_op=ALU.is_ge, fill=0.0,
                                    base=-128 * j, pattern=[[1, 512]],
                                    channel_multiplier=-1)
        for j in range(4):
            nc.scalar.copy(mask_sb[j][:], masks_f[:, j, :])
        free_masks_f()

        # ================= helpers =================
        def project_qk(tag, dst, w_sb, bcol, rhs_fn, act_only=False):
            """dst [128,2,T]: per head pair out^T = W^T @ src^T, + bias."""
            with ExitStack() as hs:
                ps = hs.enter_context(tc.tile_pool(name=f"{tag}_psqk", bufs=4, space="PSUM"))
                for tck in range(NT512):
                    psts = [ps.tile([P, 512], F32, tag="proj", name=f"{tag}_pqk{bcol}_{p}_{tck}")
                            for p in range(2)]
                    for dblk in range(8):
                        rt = rhs_fn(dblk, tck)
                        for p in range(2):
                            nc.tensor.matmul(out=psts[p][:],
                                             lhsT=w_sb[:, dblk, p * 128:(p + 1) * 128],
                                             rhs=rt, start=(dblk == 0), stop=(dblk == 7))
                    for p in range(2):
                        if p == 0 or act_only:
                            nc.scalar.activation(out=dst[:, p, tck * 512:(tck + 1) * 512],
                                                 in_=psts[p][:], func=AF.Identity,
                                                 bias=qkb_sb[:, p, bcol:bcol + 1])
                        else:
                            nc.vector.tensor_scalar_add(
                                out=dst[:, p, tck * 512:(tck + 1) * 512],
                                in0=psts[p][:], scalar1=qkb_sb[:, p, bcol:bcol + 1])

        def project_qkv_shared(tag, QT, KT, Vp, wq_sb, wk_sb, wv_sb,
                               bq_col, bk_col, src_fn, act_only=False):
            """Q (optional), K, V projections sharing one streamed pass over the
            transposed source. src_fn(dblk, tck) -> AP [128,512] fp32r.
            act_only: put all PSUM evictions on the Act engine (keeps DVE free)."""
            with ExitStack() as hs:
                ps_qk = hs.enter_context(tc.tile_pool(name=f"{tag}_psqk", bufs=2, space="PSUM"))
                ps_v = hs.enter_context(tc.tile_pool(name=f"{tag}_psv", bufs=4, space="PSUM"))
                for tck in range(NT512):
                    psq = ([ps_qk.tile([P, 512], F32, tag="q", name=f"{tag}_psq{p}_{tck}")
                            for p in range(2)] if QT is not None else None)
                    psk = [ps_qk.tile([P, 512], F32, tag="k", name=f"{tag}_psk{p}_{tck}")
                           for p in range(2)]
                    psv = [ps_v.tile([P, DKL], F32, tag="v", name=f"{tag}_psv{j}_{tck}")
                           for j in range(4)]
                    for dblk in range(8):
                        xt = src_fn(dblk, tck)
                        first, last = (dblk == 0), (dblk == 7)
                        for p in range(2):
                            if psq is not None:
                                nc.tensor.matmul(out=psq[p][:],
                                                 lhsT=wq_sb[:, dblk, p * 128:(p + 1) * 128],
                                                 rhs=xt, start=first, stop=last,
                                                 skip_group_check=True)
                            nc.tensor.matmul(out=psk[p][:],
                                             lhsT=wk_sb[:, dblk, p * 128:(p + 1) * 128],
                                             rhs=xt, start=first, stop=last,
                                             skip_group_check=True)
                        for j in range(4):
                            nc.tensor.matmul(out=psv[j][:],
                                             lhsT=xt[:, j * 128:(j + 1) * 128],
                                             rhs=wv_sb[:, dblk, :],
                                             start=first, stop=last,
                                             skip_group_check=True)
                    for p in range(2):
                        if psq is not None:
                            if p == 0 or act_only:
                                nc.scalar.activation(out=QT[:, p, tck * 512:(tck + 1) * 512],
                                                     in_=psq[p][:], func=AF.Identity,
                                                     bias=qkb_sb[:, p, bq_col:bq_col + 1])
                            else:
                                nc.vector.tensor_scalar_add(
                                    out=QT[:, p, tck * 512:(tck + 1) * 512],
                                    in0=psq[p][:], scalar1=qkb_sb[:, p, bq_col:bq_col + 1])
                        if p == 0 or act_only:
                            nc.scalar.activation(out=KT[:, p, tck * 512:(tck + 1) * 512],
                                                 in_=psk[p][:], func=AF.Identity,
                                                 bias=qkb_sb[:, p, bk_col:bk_col + 1])
                        else:
                            nc.vector.tensor_scalar_add(
                                out=KT[:, p, tck * 512:(tck + 1) * 512],
                                in0=psk[p][:], scalar1=qkb_sb[:, p, bk_col:bk_col + 1])
                    for j in range(4):
                        tb = tck * 4 + j
                        vout = Vp[:, tb, :].rearrange("p (h v) -> p h v", v=65)[:, :, 0:64]
                        vin = psv[j][:].rearrange("p (h v) -> p h v", v=64)
                        if act_only or j % 2 == 0:
                            nc.scalar.copy(vout, vin)
                        else:
                            nc.vector.tensor_copy(vout, vin)
            nc.scalar.copy(
                Vp[:].rearrange("p t (h v) -> p t h v", v=65)[:, :, :, 64:65],
                onesc_f[:, :, 0:HL, :])

        def attention_block(tag, QT, KT, Vp, attnT, causal, loc, fillers=None):
            """Attention (head pairs x q-chunks); fillers: deque of closures
            emitting independent PE work, one popped per key-block to keep the
            tensor engine's busy-streak (pstate) alive across exp waits."""
            ps_sc = loc.enter_context(tc.tile_pool(name=f"{tag}_ps_sc", bufs=2, space="PSUM"))
            ps_av = loc.enter_context(tc.tile_pool(name=f"{tag}_ps_av", bufs=1, space="PSUM"))
            sb_pt = loc.enter_context(tc.tile_pool(name=f"{tag}_pt", bufs=8))
            sb_av = loc.enter_context(tc.tile_pool(name=f"{tag}_av", bufs=3))

            for j in range(NCH):
                nkb = (j + 1) * 4 if causal else NTB
                q_sl = slice(j * 512, (j + 1) * 512)
                for p in range(2):
                    avps = [ps_av.tile([65, 512], F32, tag=f"av{h}",
                                       name=f"{tag}_avps{p}_{j}_{h}")
                            for h in range(2)]

                    def emit_av(kb, pt):
                        first, last = (kb == 0), (kb == nkb - 1)
                        for h in range(2):
                            vcol = slice((2 * p + h) * 65, (2 * p + h + 1) * 65)
                            nc.tensor.matmul(out=avps[h][:],
                                             lhsT=Vp[:, kb, vcol],
                                             rhs=pt[:, h, :], start=first,
                                             stop=last, skip_group_check=True)

                    pending = None
                    for kb in range(nkb):
                        k_sl = slice(kb * 128, (kb + 1) * 128)
                        psS = ps_sc.tile([P, 2, 512], F32, tag="sc",
                                         name=f"{tag}_sc{p}_{j}_{kb}")
                        nc.tensor.matmul(out=psS[:, 0, :], lhsT=KT[0:64, p, k_sl],
                                         rhs=QT[0:64, p, q_sl], start=True, stop=True)
                        nc.tensor.matmul(out=psS[:, 1, :], lhsT=KT[64:128, p, k_sl],
                                         rhs=QT[64:128, p, q_sl], start=True, stop=True)
                        pt = sb_pt.tile([P, 2, 512], BF16, tag="pt",
                                        name=f"{tag}_pt{p}_{j}_{kb}")
                        nc.scalar.activation(out=pt[:], in_=psS[:], func=AF.Exp,
                                             scale=0.125)
                        if causal and kb >= j * 4:
                            mj = mask_sb[kb - j * 4]
                            nc.gpsimd.tensor_mul(pt[:, 0, :], pt[:, 0, :], mj[:])
                            nc.gpsimd.tensor_mul(pt[:, 1, :], pt[:, 1, :], mj[:])
                        if fillers and (kb % 5 == 2 or (causal and kb >= j * 4 + 2)):
                            fillers.popleft()()
                        if pending is not None:
                            emit_av(*pending)
                        pending = (kb, pt)
                    emit_av(*pending)
                    if fillers:
                        fillers.popleft()()
                    for h in range(2):
                        den = sb_av.tile([1, 512], F32, tag="den0",
                                         name=f"{tag}_den0_{p}_{j}_{h}")
                        nc.vector.reciprocal(den[:], avps[h][64:65, :])
                        bc = sb_av.tile([64, 512], F32, tag="bc",
                                        name=f"{tag}_bc_{p}_{j}_{h}")
                        nc.gpsimd.partition_broadcast(bc[:], den[:], channels=64)
                        if h == 0:
                            nc.vector.tensor_mul(attnT[0:64, p, q_sl],
                                                 avps[h][0:64, :], bc[:])
                        else:
                            nc.vector.tensor_mul(attnT[64:128, p, q_sl],
                                                 avps[h][0:64, :], bc[:])

        def wo_rs(tag, attnT, wo_sb, rs_in_t, rs_out_t, loc):
            ps_y = loc.enter_context(tc.tile_pool(name=f"{tag}_psy", bufs=2, space="PSUM"))
            sb_y = loc.enter_context(tc.tile_pool(name=f"{tag}_ysb", bufs=4))
            for tb in range(NTB):
                col = tb * 128
                ysb = sb_y.tile([P, 2, 512], BF16, tag="ysb", name=f"{tag}_ysb_{tb}")
                for s in range(2):
                    psY = ps_y.tile([P, 512], F32, tag="y", name=f"{tag}_psY_{tb}_{s}")
                    for p in range(2):
                        nc.tensor.matmul(out=psY[:],
                                         lhsT=attnT[:, p, col:col + 128],
                                         rhs=wo_sb[:, p, s * 512:(s + 1) * 512],
                                         start=(p == 0), stop=(p == 1))
                    if tb % 2 == 0:
                        nc.scalar.copy(ysb[:, s, :], psY[:])
                    else:
                        nc.vector.tensor_copy(ysb[:, s, :], psY[:])
                nc.sync.dma_start(out=rs_in_t[col:col + 128, :],
                                  in_=ysb[:].rearrange("p s f -> p (s f)"))
            if with_collectives:
                nc.gpsimd.collective_compute(
                    "ReduceScatter", ALU.add, replica_groups=GROUPS,
                    ins=[rs_in_t[:]], outs=[rs_out_t[:]])

        def ln_vec(tag, j, rs_out_t, res_ap, lni, a_dst, sb_ln, gt, bt):
            """LayerNorm (vector part) of local 128-row chunk j."""
            ybf = sb_ln.tile([P, D], BF16, tag="ybf", name=f"{tag}_ybf_{j}")
            nc.sync.dma_start(out=ybf[:], in_=rs_out_t[j * P:(j + 1) * P, :])
            yown = sb_ln.tile([P, D], F32, tag="yown", name=f"{tag}_yown_{j}")
            nc.vector.tensor_add(yown[:], ybf[:], res_ap)
            ln_norm(tag, j, yown, a_dst, sb_ln, gt, bt)

        def ln_norm(tag, j, yown, a_dst, sb_ln, gt, bt):
            st = sb_ln.tile([P, 2, 6], F32, tag="st", name=f"{tag}_st_{j}")
            nc.vector.bn_stats(out=st[:, 0, :], in_=yown[:, 0:512])
            nc.vector.bn_stats(out=st[:, 1, :], in_=yown[:, 512:1024])
            mv = sb_ln.tile([P, 2], F32, tag="mv", name=f"{tag}_mv_{j}")
            nc.vector.bn_aggr(out=mv[:], in_=st[:])
            nc.scalar.activation(out=mv[:, 1:2], in_=mv[:, 1:2], func=AF.Sqrt,
                                 bias=eps_t[:])
            nc.vector.reciprocal(mv[:, 1:2], mv[:, 1:2])
            nc.vector.tensor_scalar(out=a_dst[:, j, :], in0=yown[:],
                                    scalar1=mv[:, 0:1], scalar2=mv[:, 1:2],
                                    op0=ALU.subtract, op1=ALU.mult)
            nc.vector.tensor_mul(a_dst[:, j, :], a_dst[:, j, :], gt[:])
            nc.vector.tensor_add(a_dst[:, j, :], a_dst[:, j, :], bt[:])

        def ln_transpose(tag, j, a_dst, at_dst, ps_tr):
            for dblk in range(8):
                pst = ps_tr.tile([P, P], F32, tag="tr", name=f"{tag}_tr_{j}_{dblk}")
                nc.tensor.transpose(pst[:], a_dst[:, j, dblk * 128:(dblk + 1) * 128],
                                    ident[:])
                nc.vector.tensor_copy(at_dst[:, dblk, j * 128:(j + 1) * 128], pst[:])

        # ================= phases =================
        resid = top.enter_context(tc.tile_pool(name="resid", bufs=1))
        a1_sb = resid.tile([P, 4, D], F32, name="a1_sb")
        a2_sb = resid.tile([P, 4, D], F32, name="a2_sb")
        a2t_sb = resid.tile([P, 8, TOWN], BF16, name="a2t_sb")
        lng_pool = top.enter_context(tc.tile_pool(name="lng", bufs=1))

        def emit_body(stop=None):
            with ExitStack() as ph:
                # V2 (cross attention values) is filled DURING self-attention
                # as PE filler work, so its pool spans both blocks.
                v2_pool = ph.enter_context(tc.tile_pool(name="D_v2", bufs=1))
                att2_pool = ph.enter_context(tc.tile_pool(name="D_att", bufs=1))
                V2 = v2_pool.tile([P, NTB, H * 65], BF16, name="D_V")
                attnT2 = att2_pool.tile([P, 8, TOWN], BF16, name="D_attnT")
                # --- self attention block (+ V2-projection filler) ---
                with ExitStack() as phs:
                    qkv = phs.enter_context(tc.tile_pool(name="A_qkv", bufs=1))
                    att_pool = phs.enter_context(tc.tile_pool(name="A_att", bufs=1))
                    QT1 = qkv.tile([P, 2, T], BF16, name="A_QT")
                    KT1 = qkv.tile([P, 2, T], BF16, name="A_KT")
                    V1 = qkv.tile([P, NTB, HL * 65], BF16, name="A_V")
                    attnT1 = att_pool.tile([P, 2, T], BF16, name="A_attnT")
                    with ExitStack() as loc:
                        sbw = loc.enter_context(tc.tile_pool(name="A_w", bufs=1))
                        xs_pool = loc.enter_context(tc.tile_pool(name="A_xs", bufs=8))
                        wq_sb = sbw.tile([P, 8, DKL], BF16, name="A_wq")
                        nc.sync.dma_start(out=wq_sb[:], in_=saq_w[:].rearrange("(n p) m -> p n m", p=P))
                        wk_sb = sbw.tile([P, 8, DKL], BF16, name="A_wk")
                        nc.sync.dma_start(out=wk_sb[:], in_=sak_w[:].rearrange("(n p) m -> p n m", p=P))
                        wv_sb = sbw.tile([P, 8, DKL], BF16, name="A_wv")
                        nc.sync.dma_start(out=wv_sb[:], in_=sav_w[:].rearrange("(n p) m -> p n m", p=P))

                        def x_src(dblk, tck):
                            t = xs_pool.tile([P, 512], BF16, tag="xs", name=f"A_xs_{dblk}_{tck}")
                            nc.sync.dma_start(out=t[:], in_=xT[dblk * 128:(dblk + 1) * 128,
                                                              tck * 512:(tck + 1) * 512])
                            return t[:]

                        project_qkv_shared("A", QT1, KT1, V1, wq_sb, wk_sb, wv_sb, 0, 1, x_src)
                    if stop == "qkv1":
                        return
                    with ExitStack() as loc:
                        # V2 projection filler units (enc @ Wv, all 16 heads)
                        sbwv = loc.enter_context(tc.tile_pool(name="D_wv_p", bufs=1))
                        enc_pool = loc.enter_context(tc.tile_pool(name="D_encV", bufs=3))
                        ps_v = loc.enter_context(tc.tile_pool(name="D_psv", bufs=1, space="PSUM"))
                        wv2_sb = sbwv.tile([P, 8, D], BF16, name="D_wv")
                        nc.sync.dma_start(out=wv2_sb[:], in_=cav_w[:].rearrange("(n p) m -> p n m", p=P))
                        enc_tiles = {}

                        def v_unit(tck, j):
                            def emit():
                                if tck not in enc_tiles:
                                    t = enc_pool.tile([P, 8, 512], BF16, tag="enc",
                                                      name=f"D_encV_{tck}")
                                    nc.sync.dma_start(
                                        out=t[:],
                                        in_=encT[:, tck * 512:(tck + 1) * 512]
                                        .rearrange("(n p) m -> p n m", p=P))
                                    enc_tiles[tck] = t
                                    if tck - 2 in enc_tiles:
                                        del enc_tiles[tck - 2]
                                enct = enc_tiles[tck]
                                tb = tck * 4 + j
                                psv = ps_v.tile([P, 2, 512], F32, tag="v",
                                                name=f"D_psv_{tck}_{j}")
                                for dblk in range(8):
                                    for sh in range(2):
                                        nc.tensor.matmul(
                                            out=psv[:, sh, :],
                                            lhsT=enct[:, dblk, j * 128:(j + 1) * 128],
                                            rhs=wv2_sb[:, dblk, sh * 512:(sh + 1) * 512],
                                            start=(dblk == 0), stop=(dblk == 7),
                                            skip_group_check=True)
                                vout = V2[:, tb, :].rearrange("p (h v) -> p h v", v=65)[:, :, 0:64]
                                vin = psv[:].rearrange("p s (g v) -> p (s g) v", v=64)
                                nc.vector.tensor_copy(vout, vin)
                            return emit

                        from collections import deque
                        fillers = deque(v_unit(tck, j)
                                        for tck in range(NT512) for j in range(4))
                        with ExitStack() as loc2:
                            attention_block("B", QT1, KT1, V1, attnT1, True, loc2,
                                            fillers=fillers)
                        while fillers:
                            fillers.popleft()()
                        nc.scalar.copy(
                            V2[:].rearrange("p t (h v) -> p t h v", v=65)[:, :, :, 64:65],
                            onesc_f[:])
                    if stop == "att1":
                        return
                    # Wo + single ReduceScatter (hidden under the K-pass below)
                    with ExitStack() as loc:
                        sbwo = loc.enter_context(tc.tile_pool(name="B_wo", bufs=1))
                        wo_sb = sbwo.tile([P, 2, D], BF16, name="B_wo_sb")
                        nc.sync.dma_start(out=wo_sb[:], in_=sao_w[:].rearrange("(n p) m -> p n m", p=P))
                        wo_rs("C", attnT1, wo_sb, rs_in[0], rs_out[0], loc)
                    if stop == "wo1":
                        return
                # --- cross attention (q-sharded: all 16 heads, own rows) ---
                qkv2 = ph.enter_context(tc.tile_pool(name="D_qkv", bufs=1))
                KT2 = qkv2.tile([P, 8, T], BF16, name="D_KT")
                QT2 = qkv2.tile([P, 8, TOWN], BF16, name="D_QT")
                a1t_pool = ph.enter_context(tc.tile_pool(name="a1t_pool", bufs=1))
                a1t_sb = a1t_pool.tile([P, 8, TOWN], BF16, name="a1t_sb")
                # K-pass (full heads) — overlaps the RS1 collective
                with ExitStack() as loc:
                    sbw = loc.enter_context(tc.tile_pool(name="D_wk_p", bufs=1))
                    enc_pool = loc.enter_context(tc.tile_pool(name="D_encK", bufs=2))
                    ps_k = loc.enter_context(tc.tile_pool(name="D_psk", bufs=2, space="PSUM"))
                    wk_sb = sbw.tile([P, 8, D], BF16, name="D_wk")
                    nc.sync.dma_start(out=wk_sb[:], in_=cak_w[:].rearrange("(n p) m -> p n m", p=P))
                    for tck in range(NT512):
                        enct = enc_pool.tile([P, 8, 512], BF16, tag="enc",
                                             name=f"D_encK_{tck}")
                        nc.sync.dma_start(out=enct[:],
                                          in_=encT[:, tck * 512:(tck + 1) * 512]
                                          .rearrange("(n p) m -> p n m", p=P))
                        for p in range(8):
                            psk = ps_k.tile([P, 512], F32, tag="k", name=f"D_psk_{tck}_{p}")
                            for dblk in range(8):
                                nc.tensor.matmul(out=psk[:],
                                                 lhsT=wk_sb[:, dblk, p * 128:(p + 1) * 128],
                                                 rhs=enct[:, dblk, :],
                                                 start=(dblk == 0), stop=(dblk == 7))
                            nc.scalar.activation(out=KT2[:, p, tck * 512:(tck + 1) * 512],
                                                 in_=psk[:], func=AF.Identity,
                                                 bias=cab_sb[:, p, 1:2])
                if stop == "kv2":
                    return
                # LN1 (+ residual) on own rows; transpose a1 -> a1t (bf16)
                with ExitStack() as ph2:
                    xr_pool = ph2.enter_context(tc.tile_pool(name="xr_pool", bufs=1))
                    sb_ln = ph2.enter_context(tc.tile_pool(name="C2_ln", bufs=2))
                    ps_tr = ph2.enter_context(tc.tile_pool(name="C2_ps_tr", bufs=4, space="PSUM"))
                    x_rows_sb = xr_pool.tile([P, 4, D], F32, name="x_rows_sb")
                    nc.sync.dma_start(out=x_rows_sb[:],
                                      in_=x_rows[:].rearrange("(n p) m -> p n m", p=P))
                    g1 = lng_pool.tile([P, D], F32, name="g1")
                    nc.sync.dma_start(out=g1[:], in_=ln_g[0].to_broadcast((P, D)))
                    bt1 = lng_pool.tile([P, D], F32, name="bt1")
                    nc.sync.dma_start(out=bt1[:], in_=ln_b[0].to_broadcast((P, D)))
                    for j in range(NCH):
                        ln_vec("C2", j, rs_out[0], x_rows_sb[:, j, :], 0, a1_sb,
                               sb_ln, g1, bt1)
                    for j in range(NCH):
                        ln_transpose("C2", j, a1_sb, a1t_sb, ps_tr)
                if stop == "ln1":
                    return
                # Q (all heads) from own a1^T — no collective needed
                with ExitStack() as loc:
                    sbw = loc.enter_context(tc.tile_pool(name="D_wq_p", bufs=1))
                    ps_q = loc.enter_context(tc.tile_pool(name="D_psq", bufs=2, space="PSUM"))
                    wq_sb = sbw.tile([P, 8, D], BF16, name="D_wq")
                    nc.sync.dma_start(out=wq_sb[:], in_=caq_w[:].rearrange("(n p) m -> p n m", p=P))
                    for p in range(8):
                        psq = ps_q.tile([P, 512], F32, tag="q", name=f"D_psq_{p}")
                        for dblk in range(8):
                            nc.tensor.matmul(out=psq[:],
                                             lhsT=wq_sb[:, dblk, p * 128:(p + 1) * 128],
                                             rhs=a1t_sb[:, dblk, :],
                                             start=(dblk == 0), stop=(dblk == 7))
                        nc.scalar.activation(out=QT2[:, p, :], in_=psq[:],
                                             func=AF.Identity, bias=cab_sb[:, p, 0:1])
                if stop == "qkv2":
                    return
                # q-sharded cross attention over all 16 heads
                with ExitStack() as loc:
                    ps_sc = loc.enter_context(tc.tile_pool(name="E_ps_sc", bufs=2, space="PSUM"))
                    ps_av = loc.enter_context(tc.tile_pool(name="E_ps_av", bufs=2, space="PSUM"))
                    sb_pt = loc.enter_context(tc.tile_pool(name="E_pt", bufs=6))
                    sb_av = loc.enter_context(tc.tile_pool(name="E_av", bufs=3))
                    for p in range(8):
                        avps = [ps_av.tile([65, 512], F32, tag=f"av{h}",
                                           name=f"E_avps{p}_{h}") for h in range(2)]

                        def emit_av(kb, pt):
                            first, last = (kb == 0), (kb == NTB - 1)
                            for h in range(2):
                                vcol = slice((2 * p + h) * 65, (2 * p + h + 1) * 65)
                                nc.tensor.matmul(out=avps[h][:],
                                                 lhsT=V2[:, kb, vcol],
                                                 rhs=pt[:, h, :], start=first,
                                                 stop=last, skip_group_check=True)

                        pending = None
                        for kb in range(NTB):
                            k_sl = slice(kb * 128, (kb + 1) * 128)
                            psS = ps_sc.tile([P, 2, 512], F32, tag="sc",
                                             name=f"E_sc{p}_{kb}")
                            nc.tensor.matmul(out=psS[:, 0, :], lhsT=KT2[0:64, p, k_sl],
                                             rhs=QT2[0:64, p, :], start=True, stop=True)
                            nc.tensor.matmul(out=psS[:, 1, :], lhsT=KT2[64:128, p, k_sl],
                                             rhs=QT2[64:128, p, :], start=True, stop=True)
                            pt = sb_pt.tile([P, 2, 512], BF16, tag="pt",
                                            name=f"E_pt{p}_{kb}")
                            nc.scalar.activation(out=pt[:], in_=psS[:], func=AF.Exp,
                                                 scale=0.125)
                            if pending is not None:
                                emit_av(*pending)
                            pending = (kb, pt)
                        emit_av(*pending)
                        for h in range(2):
                            den = sb_av.tile([1, 512], F32, tag="den0",
                                             name=f"E_den0_{p}_{h}")
                            nc.vector.reciprocal(den[:], avps[h][64:65, :])
                            bc = sb_av.tile([64, 512], F32, tag="bc",
                                            name=f"E_bc_{p}_{h}")
                            nc.gpsimd.partition_broadcast(bc[:], den[:], channels=64)
                            if h == 0:
                                nc.vector.tensor_mul(attnT2[0:64, p, :],
                                                     avps[h][0:64, :], bc[:])
                            else:
                                nc.vector.tensor_mul(attnT2[64:128, p, :],
                                                     avps[h][0:64, :], bc[:])
                if stop == "att2":
                    return
                # Wo (fully local contraction over all 16 heads) + LN2 fused
                with ExitStack() as ph2:
                    sbwo = ph2.enter_context(tc.tile_pool(name="F_wo", bufs=1))
                    sb_ln = ph2.enter_context(tc.tile_pool(name="F2_ln", bufs=2))
                    ps_y = ph2.enter_context(tc.tile_pool(name="F_psy", bufs=2, space="PSUM"))
                    ps_tr = ph2.enter_context(tc.tile_pool(name="F2_ps_tr", bufs=4, space="PSUM"))
                    wo_sb = sbwo.tile([P, 8, D], BF16, name="F_wo_sb")
                    nc.sync.dma_start(out=wo_sb[:], in_=cao_w[:].rearrange("(n p) m -> p n m", p=P))
                    g2 = lng_pool.tile([P, D], F32, name="g2")
                    nc.sync.dma_start(out=g2[:], in_=ln_g[1].to_broadcast((P, D)))
                    bt2 = lng_pool.tile([P, D], F32, name="bt2")
                    nc.sync.dma_start(out=bt2[:], in_=ln_b[1].to_broadcast((P, D)))
                    for tb in range(4):
                        yown = sb_ln.tile([P, D], F32, tag="yo2", name=f"F_yo2_{tb}")
                        for s in range(2):
                            psY = ps_y.tile([P, 512], F32, tag="y", name=f"F_psY_{tb}_{s}")
                            for p in range(8):
                                nc.tensor.matmul(out=psY[:],
                                                 lhsT=attnT2[:, p, tb * 128:(tb + 1) * 128],
                                                 rhs=wo_sb[:, p, s * 512:(s + 1) * 512],
                                                 start=(p == 0), stop=(p == 7))
                            if s == 0:
                                nc.scalar.copy(yown[:, 0:512], psY[:])
                            else:
                                nc.vector.tensor_copy(yown[:, 512:1024], psY[:])
                        nc.vector.tensor_add(yown[:], yown[:], a1_sb[:, tb, :])
                        ln_norm("F2", tb, yown, a2_sb, sb_ln, g2, bt2)
                    for j in range(NCH):
                        ln_transpose("F2", j, a2_sb, a2t_sb, ps_tr)
            if stop == "ln2":
                return

            # --- FFN (row-sharded) ---
            with ExitStack() as ph:
                hpool = ph.enter_context(tc.tile_pool(name="G_h", bufs=1))
                sb_ln = ph.enter_context(tc.tile_pool(name="H_ln", bufs=3))
                b1_sb = hpool.tile([P, NFB], F32, name="b1_sb")
                nc.sync.dma_start(out=b1_sb[:], in_=b1[:])
                g3 = hpool.tile([P, D], F32, name="g3")
                nc.sync.dma_start(out=g3[:], in_=ln_g[2].to_broadcast((P, D)))
                b3 = hpool.tile([P, D], F32, name="b3")
                nc.sync.dma_start(out=b3[:], in_=ln_b[2].to_broadcast((P, D)))
                hT_sb = hpool.tile([P, NFB, TOWN], BF16, name="hT_sb")
                y2_sb = hpool.tile([P, 4, D], F32, name="y2_sb")
                with ExitStack() as loc:
                    wpool = loc.enter_context(tc.tile_pool(name="G_w", bufs=2))
                    ps = loc.enter_context(tc.tile_pool(name="G_ps", bufs=3, space="PSUM"))
                    for fq in range(NFB // 4):
                        w1t = wpool.tile([P, 8, 512], BF16, tag="w1", name=f"G_w1_{fq}")
                        nc.sync.dma_start(out=w1t[:],
                                          in_=w1[:, fq * 512:(fq + 1) * 512]
                                          .rearrange("(n p) m -> p n m", p=P))
                        for fs in range(4):
                            fb = fq * 4 + fs
                            psH = ps.tile([P, 512], F32, tag="h", name=f"G_psH_{fb}")
                            for dblk in range(8):
                                nc.tensor.matmul(out=psH[:],
                                                 lhsT=w1t[:, dblk, fs * 128:(fs + 1) * 128],
                                                 rhs=a2t_sb[:, dblk, :],
                                                 start=(dblk == 0), stop=(dblk == 7))
                            nc.scalar.activation(out=hT_sb[:, fb, :], in_=psH[:], func=AF.Relu,
                                                 bias=b1_sb[:, fb:fb + 1])
                if stop == "ffn1":
                    return
                with ExitStack() as loc:
                    wpool = loc.enter_context(tc.tile_pool(name="H_w", bufs=8))
                    ps_y2 = loc.enter_context(tc.tile_pool(name="H_ps", bufs=1, space="PSUM"))
                    for s in range(2):
                        psY2 = [ps_y2.tile([P, 512], F32, tag=f"y2_{tb}",
                                           name=f"H_psY2_{s}_{tb}") for tb in range(4)]
                        for fb in range(NFB):
                            w2t = wpool.tile([P, 512], BF16, tag="w2", name=f"H_w2_{s}_{fb}")
                            nc.sync.dma_start(out=w2t[:], in_=w2[fb * 128:(fb + 1) * 128,
                                                               s * 512:(s + 1) * 512])
                            for tb in range(4):
                                nc.tensor.matmul(out=psY2[tb][:],
                                                 lhsT=hT_sb[:, fb, tb * 128:(tb + 1) * 128],
                                                 rhs=w2t[:], start=(fb == 0),
                                                 stop=(fb == NFB - 1), skip_group_check=True)
                        for tb in range(4):
                            if tb % 2 == 0:
                                nc.scalar.copy(y2_sb[:, tb, s * 512:(s + 1) * 512], psY2[tb][:])
                            else:
                                nc.vector.tensor_copy(y2_sb[:, tb, s * 512:(s + 1) * 512],
                                                      psY2[tb][:])
                for tb in range(4):
                    nc.vector.tensor_add(y2_sb[:, tb, :], y2_sb[:, tb, :], a2_sb[:, tb, :])
                    st = sb_ln.tile([P, 2, 6], F32, tag="st", name=f"H_st_{tb}")
                    nc.vector.bn_stats(out=st[:, 0, :], in_=y2_sb[:, tb, 0:512])
                    nc.vector.bn_stats(out=st[:, 1, :], in_=y2_sb[:, tb, 512:1024])
                    mv = sb_ln.tile([P, 2], F32, tag="mv", name=f"H_mv_{tb}")
                    nc.vector.bn_aggr(out=mv[:], in_=st[:])
                    nc.scalar.activation(out=mv[:, 1:2], in_=mv[:, 1:2], func=AF.Sqrt,
                                         bias=eps_t[:])
                    nc.vector.reciprocal(mv[:, 1:2], mv[:, 1:2])
                    osb = sb_ln.tile([P, D], F32, tag="osb", name=f"H_osb_{tb}")
                    nc.vector.tensor_scalar(out=osb[:], in0=y2_sb[:, tb, :],
                                            scalar1=mv[:, 0:1], scalar2=mv[:, 1:2],
                                            op0=ALU.subtract, op1=ALU.mult)
                    nc.vector.tensor_mul(osb[:], osb[:], g3[:])
                    nc.vector.tensor_add(osb[:], osb[:], b3[:])
                    nc.sync.dma_start(out=out[tb * 128:(tb + 1) * 128, :], in_=osb[:])

        for _rep in range(rep):
            emit_body(stop_after)

    nc.compile()
    return nc


_NC_CACHE = None


def _get_nc():
    global _NC_CACHE
    if _NC_CACHE is None:
        _NC_CACHE = build_kernel()
    return _NC_CACHE


def own_rows(c):
    """True row indices owned by rank c (within its batch), in local order."""
    return TOWN * c + np.arange(TOWN)


def make_in_maps(inputs):
    """Build the 8 per-core input dicts from the full problem inputs."""
    g = {k: np.asarray(v) for k, v in inputs.items()}
    la = g["lookahead_mask"]
    pm = g["padding_mask"]
    assert np.array_equal(la[0, 0], np.tril(np.ones((T, T), la.dtype))), \
        "kernel specialized for causal lookahead_mask"
    assert pm.min() == 1, "kernel specialized for all-ones padding_mask"

    r32 = round_fp32r
    # host-side bias folds (all fp64 for accuracy)
    sa_fold = (g["sa_bv"].astype(np.float64) @ g["sa_Wo"].astype(np.float64)
               + g["sa_bo"].astype(np.float64))                  # [D]
    cb = (g["ca_bv"].astype(np.float64) @ g["ca_Wo"].astype(np.float64)
          + g["ca_bo"].astype(np.float64))                       # [D]
    ln1_b = g["ln1_b"].astype(np.float64) + cb
    ln2_b = g["ln2_b"].astype(np.float64) + g["ff_b2"].astype(np.float64)
    ca_bq = g["ca_bq"].astype(np.float64) - cb @ g["ca_Wq"].astype(np.float64)
    b1_full = (g["ff_b1"].astype(np.float64)
               - g["ff_b2"].astype(np.float64) @ g["ff_W1"].astype(np.float64))

    qk_b_h = np.zeros((NC // 2, P, 2, 2), np.float32)
    for c in range(NC // 2):
        hsl = slice(DKL * c, DKL * (c + 1))
        for i, bias in enumerate((g["sa_bq"], g["sa_bk"])):
            qk_b_h[c, :, :, i] = np.asarray(bias)[hsl].reshape(2, 128).T.astype(np.float32)
    cab = np.zeros((P, 8, 2), np.float32)
    cab[:, :, 0] = ca_bq.reshape(8, 128).T.astype(np.float32)
    cab[:, :, 1] = np.asarray(g["ca_bk"]).reshape(8, 128).T.astype(np.float32)

    in_maps = []
    for r in range(NC):
        b, c = r // TPG, r % TPG
        hsl = slice(DKL * c, DKL * (c + 1))
        rows = own_rows(c)
        m = dict(
            xT=to_bf16(np.ascontiguousarray(g["x"][b].T)),
            x_rows=np.ascontiguousarray(
                g["x"][b][rows].astype(np.float64) + sa_fold).astype(np.float32),
            encT=to_bf16(np.ascontiguousarray(g["encoder_output"][b].T)),
            saq_w=to_bf16(g["sa_Wq"][:, hsl]), sak_w=to_bf16(g["sa_Wk"][:, hsl]),
            sav_w=to_bf16(g["sa_Wv"][:, hsl]),
            caq_w=to_bf16(g["ca_Wq"]),
            cak_w=to_bf16(g["ca_Wk"]), cav_w=to_bf16(g["ca_Wv"]),
            qk_b=qk_b_h[c], cab=cab,
            sao_w=to_bf16(g["sa_Wo"][hsl, :]), cao_w=to_bf16(g["ca_Wo"]),
            w1=to_bf16(g["ff_W1"]),
            b1=np.ascontiguousarray(
                b1_full.astype(np.float32).reshape(NFB, P).T),
            w2=to_bf16(g["ff_W2"]),
            ln_g=np.stack([g["ln1_g"], g["ln2_g"], g["ln3_g"]])[:, None].astype(np.float32),
            ln_b=np.stack([ln1_b.astype(np.float32), ln2_b.astype(np.float32),
                           g["ln3_b"]])[:, None].astype(np.float32),
        )
        in_maps.append(m)
    return in_maps


def assemble(results):
    outp = np.empty((B, T, D), np.float32)
    for r in range(NC):
        b, c = r // TPG, r % TPG
        outp[b][own_rows(c)] = results[r]["out"]
    return outp


def kernel(**inputs) -> np.ndarray:
    nc = _get_nc()
    in_maps = make_in_maps(inputs)
    res = run_bass_kernel_spmd(nc, in_maps, core_ids=list(range(NC)), trace=False)
    return assemble(res.results)


# revision 28
# speedup vs baseline: 1.0772x; 1.0679x over previous
"""Trainium2 Bass kernel for nn_DecoderLayer (B=2,T=2048,D=1024,H=16,dk=dv=64,dff=4096).

Sharding: 8 cores = 2 batch groups (data parallel) x 4 ranks; rank c owns
contiguous rows [512c, 512c+512) of its batch.
  - Self-attention: head-parallel (4 heads/core); S^T=[k,q] blocks; softmax
    denominator fused into AV as a ones-column of V; single AV matmul with
    full 128-key contraction. The cross-attention V projection (enc @ Wv,
    all 16 heads) is interleaved into self-attention as tensor-engine
    filler so the PE busy-streak (pstate) never drops.
  - Wo1 partials -> ONE bf16 ReduceScatter(add), hidden under the full-head
    cross K projection.
  - Cross-attention is Q-SHARDED: every rank computes all 16 heads for its
    own 512 rows, so Q2 needs only local LN1 output (no AllGather) and the
    Wo2 contraction is fully local (no second ReduceScatter).
  - FFN row-sharded, no collective. LayerNorms on own rows.
All biases are folded host-side: v-bias/Wo-bias via softmax sum-to-1 into
the LN residuals (x_rows, ln1_b, ln2_b) with Q-bias/FFN-b1 compensations.
Nearly all matmuls run in bf16 (error budget 2e-2, achieved ~1.6e-3);
residual/LN arithmetic stays fp32.
"""
from contextlib import ExitStack

import numpy as np

import concourse.bacc as bacc
import concourse.tile as tile
import concourse.mybir as mybir
from concourse.bass_utils import run_bass_kernel_spmd
from concourse.masks import make_identity

F32 = mybir.dt.float32
F32R = mybir.dt.float32r
BF16 = mybir.dt.bfloat16
AF = mybir.ActivationFunctionType
ALU = mybir.AluOpType
P = 128

B, T, D, H, DK, DV, DFF = 2, 2048, 1024, 16, 64, 64, 4096
NC, TPG = 8, 4
TOWN = T // TPG          # 512 rows owned per rank
HL = H // TPG            # 4 heads per rank
DKL = HL * DK            # 256
EPS = 1e-5
GROUPS = [[0, 1, 2, 3], [4, 5, 6, 7]]
NT512 = T // 512         # 4
NTB = T // P             # 16
NFB = DFF // P           # 32
NCH = 4                  # local 128-row LayerNorm chunks


def round_fp32r(x: np.ndarray) -> np.ndarray:
    u = np.ascontiguousarray(x, dtype=np.float32).view(np.uint32)
    return ((u.astype(np.uint64) + 0x800) & 0xFFFFF000).astype(np.uint32).view(np.float32)


def to_bf16(x: np.ndarray) -> np.ndarray:
    return np.ascontiguousarray(np.asarray(x)).astype(mybir.dt.np(BF16))


def build_kernel(with_collectives=True, rep=1, stop_after=None):
    nc = bacc.Bacc("TRN2", target_bir_lowering=False, num_devices=NC)
    with tile.TileContext(nc) as tc, ExitStack() as top:
        dram = top.enter_context(tc.tile_pool(name="dram", bufs=1, space="DRAM"))

        def din(name, shape, dtype=F32R):
            return dram.tile(shape, dtype, kind="ExternalInput", uniquify=False, name=name)

        # ---------- I/O ----------
        xT = din("xT", [D, T], BF16)
        x_rows = din("x_rows", [TOWN, D], F32)      # strided-own rows + folds
        encT = din("encT", [D, T], BF16)
        saq_w = din("saq_w", [D, DKL], BF16); sak_w = din("sak_w", [D, DKL], BF16); sav_w = din("sav_w", [D, DKL], BF16)
        caq_w = din("caq_w", [D, D], BF16)
        cak_w = din("cak_w", [D, D], BF16); cav_w = din("cav_w", [D, D], BF16)
        qk_b = din("qk_b", [P, 2, 2], F32)          # [part, pair, (saq,sak)]
        cab = din("cab", [P, 8, 2], F32)            # [part, pair, (caq,cak)]
        sao_w = din("sao_w", [DKL, D], BF16); cao_w = din("cao_w", [D, D], BF16)
        w1 = din("w1", [D, DFF], BF16); b1 = din("b1", [P, NFB], F32)
        w2 = din("w2", [DFF, D], BF16)
        ln_g = din("ln_g", [3, 1, D], F32); ln_b = din("ln_b", [3, 1, D], F32)
        out = dram.tile([TOWN, D], F32, kind="ExternalOutput", uniquify=False, name="out")

        rs_in = [dram.tile([T, D], BF16, name="rs0_in")]
        rs_out = [dram.tile([TOWN, D], BF16, name="rs0_out")]

        # ---------- persistent SBUF ----------
        const = top.enter_context(tc.tile_pool(name="const", bufs=1))
        ident = const.tile([P, P], F32, name="ident")
        make_identity(nc, ident)
        eps_t = const.tile([P, 1], F32, name="eps_t")
        nc.vector.memset(eps_t[:], EPS)
        onesc_f = const.tile([P, NTB, H, 1], F32, name="onesc_f")
        nc.vector.memset(onesc_f[:], 1.0)

        qkb_sb = const.tile([P, 2, 2], F32, name="qkb_sb")
        nc.sync.dma_start(out=qkb_sb[:], in_=qk_b[:])
        cab_sb = const.tile([P, 8, 2], F32, name="cab_sb")
        nc.sync.dma_start(out=cab_sb[:], in_=cab[:])

        # causal diagonal masks: mask_j[k,q] = 1 if (q - 128*j - k) >= 0
        mask_sb = [const.tile([P, 512], BF16, name=f"mask_sb{j}") for j in range(4)]
        masks_f, free_masks_f = tc.tile([P, 4, 512], F32, name="masks_f")
        nc.gpsimd.memset(masks_f[:], 1.0)
        for j in range(4):
            nc.gpsimd.affine_select(out=masks_f[:, j, :], in_=masks_f[:, j, :],
                                    compare_op=ALU.is_ge, fill=0.0,
                                    base=-128 * j, pattern=[[1, 512]],
                                    channel_multiplier=-1)
        for j in range(4):
            nc.scalar.copy(mask_sb[j][:], masks_f[:, j, :])
        free_masks_f()

        # ================= helpers =================
        def project_qk(tag, dst, w_sb, bcol, rhs_fn, act_only=False):
            """dst [128,2,T]: per head pair out^T = W^T @ src^T, + bias."""
            with ExitStack() as hs:
                ps = hs.enter_context(tc.tile_pool(name=f"{tag}_psqk", bufs=4, space="PSUM"))
                for tck in range(NT512):
                    psts = [ps.tile([P, 512], F32, tag="proj", name=f"{tag}_pqk{bcol}_{p}_{tck}")
                            for p in range(2)]
                    for dblk in range(8):
                        rt = rhs_fn(dblk, tck)
                        for p in range(2):
                            nc.tensor.matmul(out=psts[p][:],
                                             lhsT=w_sb[:, dblk, p * 128:(p + 1) * 128],
                                             rhs=rt, start=(dblk == 0), stop=(dblk == 7))
                    for p in range(2):
                        if p == 0 or act_only:
                            nc.scalar.activation(out=dst[:, p, tck * 512:(tck + 1) * 512],
                                                 in_=psts[p][:], func=AF.Identity,
                                                 bias=qkb_sb[:, p, bcol:bcol + 1])
                        else:
                            nc.vector.tensor_scalar_add(
                                out=dst[:, p, tck * 512:(tck + 1) * 512],
                                in0=psts[p][:], scalar1=qkb_sb[:, p, bcol:bcol + 1])

        def project_qkv_shared(tag, QT, KT, Vp, wq_sb, wk_sb, wv_sb,
                               bq_col, bk_col, src_fn, act_only=False):
            """Q (optional), K, V projections sharing one streamed pass over the
            transposed source. src_fn(dblk, tck) -> AP [128,512] fp32r.
            act_only: put all PSUM evictions on the Act engine (keeps DVE free)."""
            with ExitStack() as hs:
                ps_qk = hs.enter_context(tc.tile_pool(name=f"{tag}_psqk", bufs=2, space="PSUM"))
                ps_v = hs.enter_context(tc.tile_pool(name=f"{tag}_psv", bufs=4, space="PSUM"))
                for tck in range(NT512):
                    psq = ([ps_qk.tile([P, 512], F32, tag="q", name=f"{tag}_psq{p}_{tck}")
                            for p in range(2)] if QT is not None else None)
                    psk = [ps_qk.tile([P, 512], F32, tag="k", name=f"{tag}_psk{p}_{tck}")
                           for p in range(2)]
                    psv = [ps_v.tile([P, DKL], F32, tag="v", name=f"{tag}_psv{j}_{tck}")
                           for j in range(4)]
                    for dblk in range(8):
                        xt = src_fn(dblk, tck)
                        first, last = (dblk == 0), (dblk == 7)
                        for p in range(2):
                            if psq is not None:
                                nc.tensor.matmul(out=psq[p][:],
                                                 lhsT=wq_sb[:, dblk, p * 128:(p + 1) * 128],
                                                 rhs=xt, start=first, stop=last,
                                                 skip_group_check=True)
                            nc.tensor.matmul(out=psk[p][:],
                                             lhsT=wk_sb[:, dblk, p * 128:(p + 1) * 128],
                                             rhs=xt, start=first, stop=last,
                                             skip_group_check=True)
                        for j in range(4):
                            nc.tensor.matmul(out=psv[j][:],
                                             lhsT=xt[:, j * 128:(j + 1) * 128],
                                             rhs=wv_sb[:, dblk, :],
                                             start=first, stop=last,
                                             skip_group_check=True)
                    for p in range(2):
                        if psq is not None:
                            if p == 0 or act_only:
                                nc.scalar.activation(out=QT[:, p, tck * 512:(tck + 1) * 512],
                                                     in_=psq[p][:], func=AF.Identity,
                                                     bias=qkb_sb[:, p, bq_col:bq_col + 1])
                            else:
                                nc.vector.tensor_scalar_add(
                                    out=QT[:, p, tck * 512:(tck + 1) * 512],
                                    in0=psq[p][:], scalar1=qkb_sb[:, p, bq_col:bq_col + 1])
                        if p == 0 or act_only:
                            nc.scalar.activation(out=KT[:, p, tck * 512:(tck + 1) * 512],
                                                 in_=psk[p][:], func=AF.Identity,
                                                 bias=qkb_sb[:, p, bk_col:bk_col + 1])
                        else:
                            nc.vector.tensor_scalar_add(
                                out=KT[:, p, tck * 512:(tck + 1) * 512],
                                in0=psk[p][:], scalar1=qkb_sb[:, p, bk_col:bk_col + 1])
                    for j in range(4):
                        tb = tck * 4 + j
                        vout = Vp[:, tb, :].rearrange("p (h v) -> p h v", v=65)[:, :, 0:64]
                        vin = psv[j][:].rearrange("p (h v) -> p h v", v=64)
                        if act_only or j % 2 == 0:
                            nc.scalar.copy(vout, vin)
                        else:
                            nc.vector.tensor_copy(vout, vin)
            nc.scalar.copy(
                Vp[:].rearrange("p t (h v) -> p t h v", v=65)[:, :, :, 64:65],
                onesc_f[:, :, 0:HL, :])

        def attention_block(tag, QT, KT, Vp, attnT, causal, loc, fillers=None):
            """Attention (head pairs x q-chunks); fillers: deque of closures
            emitting independent PE work, one popped per key-block to keep the
            tensor engine's busy-streak (pstate) alive across exp waits."""
            ps_sc = loc.enter_context(tc.tile_pool(name=f"{tag}_ps_sc", bufs=2, space="PSUM"))
            ps_av = loc.enter_context(tc.tile_pool(name=f"{tag}_ps_av", bufs=1, space="PSUM"))
            sb_pt = loc.enter_context(tc.tile_pool(name=f"{tag}_pt", bufs=8))
            sb_av = loc.enter_context(tc.tile_pool(name=f"{tag}_av", bufs=3))

            for j in range(NCH):
                nkb = (j + 1) * 4 if causal else NTB
                q_sl = slice(j * 512, (j + 1) * 512)
                for p in range(2):
                    avps = [ps_av.tile([65, 512], F32, tag=f"av{h}",
                                       name=f"{tag}_avps{p}_{j}_{h}")
                            for h in range(2)]

                    def emit_av(kb, pt):
                        first, last = (kb == 0), (kb == nkb - 1)
                        for h in range(2):
                            vcol = slice((2 * p + h) * 65, (2 * p + h + 1) * 65)
                            nc.tensor.matmul(out=avps[h][:],
                                             lhsT=Vp[:, kb, vcol],
                                             rhs=pt[:, h, :], start=first,
                                             stop=last, skip_group_check=True)

                    pending = None
                    for kb in range(nkb):
                        k_sl = slice(kb * 128, (kb + 1) * 128)
                        psS = ps_sc.tile([P, 2, 512], F32, tag="sc",
                                         name=f"{tag}_sc{p}_{j}_{kb}")
                        nc.tensor.matmul(out=psS[:, 0, :], lhsT=KT[0:64, p, k_sl],
                                         rhs=QT[0:64, p, q_sl], start=True, stop=True)
                        nc.tensor.matmul(out=psS[:, 1, :], lhsT=KT[64:128, p, k_sl],
                                         rhs=QT[64:128, p, q_sl], start=True, stop=True)
                        pt = sb_pt.tile([P, 2, 512], BF16, tag="pt",
                                        name=f"{tag}_pt{p}_{j}_{kb}")
                        nc.scalar.activation(out=pt[:], in_=psS[:], func=AF.Exp,
                                             scale=0.125)
                        if causal and kb >= j * 4:
                            mj = mask_sb[kb - j * 4]
                            nc.gpsimd.tensor_mul(pt[:, 0, :], pt[:, 0, :], mj[:])
                            nc.gpsimd.tensor_mul(pt[:, 1, :], pt[:, 1, :], mj[:])
                        if fillers and (kb % 5 == 2 or (causal and kb >= j * 4 + 2)):
                            fillers.popleft()()
                        if pending is not None:
                            emit_av(*pending)
                        pending = (kb, pt)
                    emit_av(*pending)
                    if fillers:
                        fillers.popleft()()
                    for h in range(2):
                        den = sb_av.tile([1, 512], F32, tag="den0",
                                         name=f"{tag}_den0_{p}_{j}_{h}")
                        nc.vector.reciprocal(den[:], avps[h][64:65, :])
                        bc = sb_av.tile([64, 512], F32, tag="bc",
                                        name=f"{tag}_bc_{p}_{j}_{h}")
                        nc.gpsimd.partition_broadcast(bc[:], den[:], channels=64)
                        if h == 0:
                            nc.vector.tensor_mul(attnT[0:64, p, q_sl],
                                                 avps[h][0:64, :], bc[:])
                        else:
                            nc.vector.tensor_mul(attnT[64:128, p, q_sl],
                                                 avps[h][0:64, :], bc[:])

        def wo_rs(tag, attnT, wo_sb, rs_in_t, rs_out_t, loc):
            ps_y = loc.enter_context(tc.tile_pool(name=f"{tag}_psy", bufs=2, space="PSUM"))
            sb_y = loc.enter_context(tc.tile_pool(name=f"{tag}_ysb", bufs=4))
            for tb in range(NTB):
                col = tb * 128
                ysb = sb_y.tile([P, 2, 512], BF16, tag="ysb", name=f"{tag}_ysb_{tb}")
                for s in range(2):
                    psY = ps_y.tile([P, 512], F32, tag="y", name=f"{tag}_psY_{tb}_{s}")
                    for p in range(2):
                        nc.tensor.matmul(out=psY[:],
                                         lhsT=attnT[:, p, col:col + 128],
                                         rhs=wo_sb[:, p, s * 512:(s + 1) * 512],
                                         start=(p == 0), stop=(p == 1))
                    if tb % 2 == 0:
                        nc.scalar.copy(ysb[:, s, :], psY[:])
                    else:
                        nc.vector.tensor_copy(ysb[:, s, :], psY[:])
                nc.sync.dma_start(out=rs_in_t[col:col + 128, :],
                                  in_=ysb[:].rearrange("p s f -> p (s f)"))
            if with_collectives:
                nc.gpsimd.collective_compute(
                    "ReduceScatter", ALU.add, replica_groups=GROUPS,
                    ins=[rs_in_t[:]], outs=[rs_out_t[:]])

        def ln_vec(tag, j, rs_out_t, res_ap, lni, a_dst, sb_ln, gt, bt):
            """LayerNorm (vector part) of local 128-row chunk j."""
            ybf = sb_ln.tile([P, D], BF16, tag="ybf", name=f"{tag}_ybf_{j}")
            nc.sync.dma_start(out=ybf[:], in_=rs_out_t[j * P:(j + 1) * P, :])
            yown = sb_ln.tile([P, D], F32, tag="yown", name=f"{tag}_yown_{j}")
            nc.vector.tensor_add(yown[:], ybf[:], res_ap)
            ln_norm(tag, j, yown, a_dst, sb_ln, gt, bt)

        def ln_norm(tag, j, yown, a_dst, sb_ln, gt, bt):
            st = sb_ln.tile([P, 2, 6], F32, tag="st", name=f"{tag}_st_{j}")
            nc.vector.bn_stats(out=st[:, 0, :], in_=yown[:, 0:512])
            nc.vector.bn_stats(out=st[:, 1, :], in_=yown[:, 512:1024])
            mv = sb_ln.tile([P, 2], F32, tag="mv", name=f"{tag}_mv_{j}")
            nc.vector.bn_aggr(out=mv[:], in_=st[:])
            nc.scalar.activation(out=mv[:, 1:2], in_=mv[:, 1:2], func=AF.Sqrt,
                                 bias=eps_t[:])
            nc.vector.reciprocal(mv[:, 1:2], mv[:, 1:2])
            nc.vector.tensor_scalar(out=a_dst[:, j, :], in0=yown[:],
                                    scalar1=mv[:, 0:1], scalar2=mv[:, 1:2],
                                    op0=ALU.subtract, op1=ALU.mult)
            nc.vector.tensor_mul(a_dst[:, j, :], a_dst[:, j, :], gt[:])
            nc.vector.tensor_add(a_dst[:, j, :], a_dst[:, j, :], bt[:])

        def ln_transpose(tag, j, a_dst, at_dst, ps_tr):
            for dblk in range(8):
                pst = ps_tr.tile([P, P], F32, tag="tr", name=f"{tag}_tr_{j}_{dblk}")
                nc.tensor.transpose(pst[:], a_dst[:, j, dblk * 128:(dblk + 1) * 128],
                                    ident[:])
                nc.vector.tensor_copy(at_dst[:, dblk, j * 128:(j + 1) * 128], pst[:])

        # ================= phases =================
        resid = top.enter_context(tc.tile_pool(name="resid", bufs=1))
        a1_sb = resid.tile([P, 4, D], F32, name="a1_sb")
        a2_sb = resid.tile([P, 4, D], F32, name="a2_sb")
        a2t_sb = resid.tile([P, 8, TOWN], BF16, name="a2t_sb")
        lng_pool = top.enter_context(tc.tile_pool(name="lng", bufs=1))

        def emit_body(stop=None):
            with ExitStack() as ph:
                # V2 (cross attention values) is filled DURING self-attention
                # as PE filler work, so its pool spans both blocks.
                v2_pool = ph.enter_context(tc.tile_pool(name="D_v2", bufs=1))
                att2_pool = ph.enter_context(tc.tile_pool(name="D_att", bufs=1))
                V2 = v2_pool.tile([P, NTB, H * 65], BF16, name="D_V")
                attnT2 = att2_pool.tile([P, 8, TOWN], BF16, name="D_attnT")
                # --- self attention block (+ V2-projection filler) ---
                with ExitStack() as phs:
                    qkv = phs.enter_context(tc.tile_pool(name="A_qkv", bufs=1))
                    att_pool = phs.enter_context(tc.tile_pool(name="A_att", bufs=1))
                    QT1 = qkv.tile([P, 2, T], BF16, name="A_QT")
                    KT1 = qkv.tile([P, 2, T], BF16, name="A_KT")
                    V1 = qkv.tile([P, NTB, HL * 65], BF16, name="A_V")
                    attnT1 = att_pool.tile([P, 2, T], BF16, name="A_attnT")
                    with ExitStack() as loc:
                        sbw = loc.enter_context(tc.tile_pool(name="A_w", bufs=1))
                        xs_pool = loc.enter_context(tc.tile_pool(name="A_xs", bufs=8))
                        wq_sb = sbw.tile([P, 8, DKL], BF16, name="A_wq")
                        nc.sync.dma_start(out=wq_sb[:], in_=saq_w[:].rearrange("(n p) m -> p n m", p=P))
                        wk_sb = sbw.tile([P, 8, DKL], BF16, name="A_wk")
                        nc.sync.dma_start(out=wk_sb[:], in_=sak_w[:].rearrange("(n p) m -> p n m", p=P))
                        wv_sb = sbw.tile([P, 8, DKL], BF16, name="A_wv")
                        nc.sync.dma_start(out=wv_sb[:], in_=sav_w[:].rearrange("(n p) m -> p n m", p=P))

                        def x_src(dblk, tck):
                            t = xs_pool.tile([P, 512], BF16, tag="xs", name=f"A_xs_{dblk}_{tck}")
                            nc.sync.dma_start(out=t[:], in_=xT[dblk * 128:(dblk + 1) * 128,
                                                              tck * 512:(tck + 1) * 512])
                            return t[:]

                        project_qkv_shared("A", QT1, KT1, V1, wq_sb, wk_sb, wv_sb, 0, 1, x_src)
                    if stop == "qkv1":
                        return
                    with ExitStack() as loc:
                        # V2 projection filler units (enc @ Wv, all 16 heads)
                        sbwv = loc.enter_context(tc.tile_pool(name="D_wv_p", bufs=1))
                        enc_pool = loc.enter_context(tc.tile_pool(name="D_encV", bufs=3))
                        ps_v = loc.enter_context(tc.tile_pool(name="D_psv", bufs=1, space="PSUM"))
                        wv2_sb = sbwv.tile([P, 8, D], BF16, name="D_wv")
                        nc.sync.dma_start(out=wv2_sb[:], in_=cav_w[:].rearrange("(n p) m -> p n m", p=P))
                        enc_tiles = {}

                        def v_unit(tck, j):
                            def emit():
                                if tck not in enc_tiles:
                                    t = enc_pool.tile([P, 8, 512], BF16, tag="enc",
                                                      name=f"D_encV_{tck}")
                                    nc.sync.dma_start(
                                        out=t[:],
                                        in_=encT[:, tck * 512:(tck + 1) * 512]
                                        .rearrange("(n p) m -> p n m", p=P))
                                    enc_tiles[tck] = t
                                    if tck - 2 in enc_tiles:
                                        del enc_tiles[tck - 2]
                                enct = enc_tiles[tck]
                                tb = tck * 4 + j
                                psv = ps_v.tile([P, 2, 512], F32, tag="v",
                                                name=f"D_psv_{tck}_{j}")
                                for dblk in range(8):
                                    for sh in range(2):
                                        nc.tensor.matmul(
                                            out=psv[:, sh, :],
                                            lhsT=enct[:, dblk, j * 128:(j + 1) * 128],
                                            rhs=wv2_sb[:, dblk, sh * 512:(sh + 1) * 512],
                                            start=(dblk == 0), stop=(dblk == 7),
                                            skip_group_check=True)
                                vout = V2[:, tb, :].rearrange("p (h v) -> p h v", v=65)[:, :, 0:64]
                                vin = psv[:].rearrange("p s (g v) -> p (s g) v", v=64)
                                nc.vector.tensor_copy(vout, vin)
                            return emit

                        from collections import deque
                        fillers = deque(v_unit(tck, j)
                                        for tck in range(NT512) for j in range(4))
                        with ExitStack() as loc2:
                            attention_block("B", QT1, KT1, V1, attnT1, True, loc2,
                                            fillers=fillers)
                        while fillers:
                            fillers.popleft()()
                        nc.scalar.copy(
                            V2[:].rearrange("p t (h v) -> p t h v", v=65)[:, :, :, 64:65],
                            onesc_f[:])
                    if stop == "att1":
                        return
                    # Wo + single ReduceScatter (hidden under the K-pass below)
                    with ExitStack() as loc:
                        sbwo = loc.enter_context(tc.tile_pool(name="B_wo", bufs=1))
                        wo_sb = sbwo.tile([P, 2, D], BF16, name="B_wo_sb")
                        nc.sync.dma_start(out=wo_sb[:], in_=sao_w[:].rearrange("(n p) m -> p n m", p=P))
                        wo_rs("C", attnT1, wo_sb, rs_in[0], rs_out[0], loc)
                    if stop == "wo1":
                        return
                # --- cross attention (q-sharded: all 16 heads, own rows) ---
                qkv2 = ph.enter_context(tc.tile_pool(name="D_qkv", bufs=1))
                KT2 = qkv2.tile([P, 8, T], BF16, name="D_KT")
                QT2 = qkv2.tile([P, 8, TOWN], BF16, name="D_QT")
                a1t_pool = ph.enter_context(tc.tile_pool(name="a1t_pool", bufs=1))
                a1t_sb = a1t_pool.tile([P, 8, TOWN], BF16, name="a1t_sb")
                # K-pass (full heads) — overlaps the RS1 collective
                with ExitStack() as loc:
                    sbw = loc.enter_context(tc.tile_pool(name="D_wk_p", bufs=1))
                    enc_pool = loc.enter_context(tc.tile_pool(name="D_encK", bufs=2))
                    ps_k = loc.enter_context(tc.tile_pool(name="D_psk", bufs=2, space="PSUM"))
                    wk_sb = sbw.tile([P, 8, D], BF16, name="D_wk")
                    nc.sync.dma_start(out=wk_sb[:], in_=cak_w[:].rearrange("(n p) m -> p n m", p=P))
                    for tck in range(NT512):
                        enct = enc_pool.tile([P, 8, 512], BF16, tag="enc",
                                             name=f"D_encK_{tck}")
                        nc.sync.dma_start(out=enct[:],
                                          in_=encT[:, tck * 512:(tck + 1) * 512]
                                          .rearrange("(n p) m -> p n m", p=P))
                        for p in range(8):
                            psk = ps_k.tile([P, 512], F32, tag="k", name=f"D_psk_{tck}_{p}")
                            for dblk in range(8):
                                nc.tensor.matmul(out=psk[:],
                                                 lhsT=wk_sb[:, dblk, p * 128:(p + 1) * 128],
                                                 rhs=enct[:, dblk, :],
                                                 start=(dblk == 0), stop=(dblk == 7))
                            nc.scalar.activation(out=KT2[:, p, tck * 512:(tck + 1) * 512],
                                                 in_=psk[:], func=AF.Identity,
                                                 bias=cab_sb[:, p, 1:2])
                if stop == "kv2":
                    return
                # LN1 (+ residual) on own rows; transpose a1 -> a1t (bf16)
                with ExitStack() as ph2:
                    xr_pool = ph2.enter_context(tc.tile_pool(name="xr_pool", bufs=1))
                    sb_ln = ph2.enter_context(tc.tile_pool(name="C2_ln", bufs=2))
                    ps_tr = ph2.enter_context(tc.tile_pool(name="C2_ps_tr", bufs=4, space="PSUM"))
                    x_rows_sb = xr_pool.tile([P, 4, D], F32, name="x_rows_sb")
                    nc.sync.dma_start(out=x_rows_sb[:],
                                      in_=x_rows[:].rearrange("(n p) m -> p n m", p=P))
                    g1 = lng_pool.tile([P, D], F32, name="g1")
                    nc.sync.dma_start(out=g1[:], in_=ln_g[0].to_broadcast((P, D)))
                    bt1 = lng_pool.tile([P, D], F32, name="bt1")
                    nc.sync.dma_start(out=bt1[:], in_=ln_b[0].to_broadcast((P, D)))
                    for j in range(NCH):
                        ln_vec("C2", j, rs_out[0], x_rows_sb[:, j, :], 0, a1_sb,
                               sb_ln, g1, bt1)
                    for j in range(NCH):
                        ln_transpose("C2", j, a1_sb, a1t_sb, ps_tr)
                if stop == "ln1":
                    return
                # Q (all heads) from own a1^T — no collective needed
                with ExitStack() as loc:
                    sbw = loc.enter_context(tc.tile_pool(name="D_wq_p", bufs=1))
                    ps_q = loc.enter_context(tc.tile_pool(name="D_psq", bufs=2, space="PSUM"))
                    wq_sb = sbw.tile([P, 8, D], BF16, name="D_wq")
                    nc.sync.dma_start(out=wq_sb[:], in_=caq_w[:].rearrange("(n p) m -> p n m", p=P))
                    for p in range(8):
                        psq = ps_q.tile([P, 512], F32, tag="q", name=f"D_psq_{p}")
                        for dblk in range(8):
                            nc.tensor.matmul(out=psq[:],
                                             lhsT=wq_sb[:, dblk, p * 128:(p + 1) * 128],
                                             rhs=a1t_sb[:, dblk, :],
                                             start=(dblk == 0), stop=(dblk == 7))
                        nc.scalar.activation(out=QT2[:, p, :], in_=psq[:],
                                             func=AF.Identity, bias=cab_sb[:, p, 0:1])
                if stop == "qkv2":
                    return
                # q-sharded cross attention over all 16 heads
                with ExitStack() as loc:
                    ps_sc = loc.enter_context(tc.tile_pool(name="E_ps_sc", bufs=2, space="PSUM"))
                    ps_av = loc.enter_context(tc.tile_pool(name="E_ps_av", bufs=2, space="PSUM"))
                    sb_pt = loc.enter_context(tc.tile_pool(name="E_pt", bufs=6))
                    sb_av = loc.enter_context(tc.tile_pool(name="E_av", bufs=3))
                    for p in range(8):
                        avps = [ps_av.tile([65, 512], F32, tag=f"av{h}",
                                           name=f"E_avps{p}_{h}") for h in range(2)]

                        def emit_av(kb, pt):
                            first, last = (kb == 0), (kb == NTB - 1)
                            for h in range(2):
                                vcol = slice((2 * p + h) * 65, (2 * p + h + 1) * 65)
                                nc.tensor.matmul(out=avps[h][:],
                                                 lhsT=V2[:, kb, vcol],
                                                 rhs=pt[:, h, :], start=first,
                                                 stop=last, skip_group_check=True)

                        pending = None
                        for kb in range(NTB):
                            k_sl = slice(kb * 128, (kb + 1) * 128)
                            psS = ps_sc.tile([P, 2, 512], F32, tag="sc",
                                             name=f"E_sc{p}_{kb}")
                            nc.tensor.matmul(out=psS[:, 0, :], lhsT=KT2[0:64, p, k_sl],
                                             rhs=QT2[0:64, p, :], start=True, stop=True)
                            nc.tensor.matmul(out=psS[:, 1, :], lhsT=KT2[64:128, p, k_sl],
                                             rhs=QT2[64:128, p, :], start=True, stop=True)
                            pt = sb_pt.tile([P, 2, 512], BF16, tag="pt",
                                            name=f"E_pt{p}_{kb}")
                            nc.scalar.activation(out=pt[:], in_=psS[:], func=AF.Exp,
                                                 scale=0.125)
                            if pending is not None:
                                emit_av(*pending)
                            pending = (kb, pt)
                        emit_av(*pending)
                        for h in range(2):
                            den = sb_av.tile([1, 512], F32, tag="den0",
                                             name=f"E_den0_{p}_{h}")
                            nc.vector.reciprocal(den[:], avps[h][64:65, :])
                            bc = sb_av.tile([64, 512], F32, tag="bc",
                                            name=f"E_bc_{p}_{h}")
                            nc.gpsimd.partition_broadcast(bc[:], den[:], channels=64)
                            if h == 0:
                                nc.vector.tensor_mul(attnT2[0:64, p, :],
                                                     avps[h][0:64, :], bc[:])
                            else:
                                nc.vector.tensor_mul(attnT2[64:128, p, :],
                                                     avps[h][0:64, :], bc[:])
                if stop == "att2":
                    return
                # Wo (fully local contraction over all 16 heads) + LN2 fused
                with ExitStack() as ph2:
                    sbwo = ph2.enter_context(tc.tile_pool(name="F_wo", bufs=1))
                    sb_ln = ph2.enter_context(tc.tile_pool(name="F2_ln", bufs=2))
                    ps_y = ph2.enter_context(tc.tile_pool(name="F_psy", bufs=2, space="PSUM"))
                    ps_tr = ph2.enter_context(tc.tile_pool(name="F2_ps_tr", bufs=4, space="PSUM"))
                    wo_sb = sbwo.tile([P, 8, D], BF16, name="F_wo_sb")
                    nc.sync.dma_start(out=wo_sb[:], in_=cao_w[:].rearrange("(n p) m -> p n m", p=P))
                    g2 = lng_pool.tile([P, D], F32, name="g2")
                    nc.sync.dma_start(out=g2[:], in_=ln_g[1].to_broadcast((P, D)))
                    bt2 = lng_pool.tile([P, D], F32, name="bt2")
                    nc.sync.dma_start(out=bt2[:], in_=ln_b[1].to_broadcast((P, D)))
                    for tb in range(4):
                        yown = sb_ln.tile([P, D], F32, tag="yo2", name=f"F_yo2_{tb}")
                        for s in range(2):
                            psY = ps_y.tile([P, 512], F32, tag="y", name=f"F_psY_{tb}_{s}")
                            for p in range(8):
                                nc.tensor.matmul(out=psY[:],
                                                 lhsT=attnT2[:, p, tb * 128:(tb + 1) * 128],
                                                 rhs=wo_sb[:, p, s * 512:(s + 1) * 512],
                                                 start=(p == 0), stop=(p == 7))
                            if s == 0:
                                nc.scalar.copy(yown[:, 0:512], psY[:])
                            else:
                                nc.vector.tensor_copy(yown[:, 512:1024], psY[:])
                        nc.vector.tensor_add(yown[:], yown[:], a1_sb[:, tb, :])
                        ln_norm("F2", tb, yown, a2_sb, sb_ln, g2, bt2)
                    for j in range(NCH):
                        ln_transpose("F2", j, a2_sb, a2t_sb, ps_tr)
            if stop == "ln2":
                return

            # --- FFN (row-sharded) ---
            with ExitStack() as ph:
                hpool = ph.enter_context(tc.tile_pool(name="G_h", bufs=1))
                sb_ln = ph.enter_context(tc.tile_pool(name="H_ln", bufs=3))
                b1_sb = hpool.tile([P, NFB], F32, name="b1_sb")
                nc.sync.dma_start(out=b1_sb[:], in_=b1[:])
                g3 = hpool.tile([P, D], F32, name="g3")
                nc.sync.dma_start(out=g3[:], in_=ln_g[2].to_broadcast((P, D)))
                b3 = hpool.tile([P, D], F32, name="b3")
                nc.sync.dma_start(out=b3[:], in_=ln_b[2].to_broadcast((P, D)))
                hT_sb = hpool.tile([P, NFB, TOWN], BF16, name="hT_sb")
                y2_sb = hpool.tile([P, 4, D], F32, name="y2_sb")
                with ExitStack() as loc:
                    wpool = loc.enter_context(tc.tile_pool(name="G_w", bufs=3))
                    ps = loc.enter_context(tc.tile_pool(name="G_ps", bufs=3, space="PSUM"))
                    for fq in range(NFB // 4):
                        w1t = wpool.tile([P, 8, 512], BF16, tag="w1", name=f"G_w1_{fq}")
                        nc.sync.dma_start(out=w1t[:],
                                          in_=w1[:, fq * 512:(fq + 1) * 512]
                                          .rearrange("(n p) m -> p n m", p=P))
                        for fs in range(4):
                            fb = fq * 4 + fs
                            psH = ps.tile([P, 512], F32, tag="h", name=f"G_psH_{fb}")
                            for dblk in range(8):
                                nc.tensor.matmul(out=psH[:],
                                                 lhsT=w1t[:, dblk, fs * 128:(fs + 1) * 128],
                                                 rhs=a2t_sb[:, dblk, :],
                                                 start=(dblk == 0), stop=(dblk == 7))
                            nc.scalar.activation(out=hT_sb[:, fb, :], in_=psH[:], func=AF.Relu,
                                                 bias=b1_sb[:, fb:fb + 1])
                if stop == "ffn1":
                    return
                with ExitStack() as loc:
                    wpool = loc.enter_context(tc.tile_pool(name="H_w", bufs=12))
                    ps_y2 = loc.enter_context(tc.tile_pool(name="H_ps", bufs=1, space="PSUM"))
                    for s in range(2):
                        psY2 = [ps_y2.tile([P, 512], F32, tag=f"y2_{tb}",
                                           name=f"H_psY2_{s}_{tb}") for tb in range(4)]
                        for fb in range(NFB):
                            w2t = wpool.tile([P, 512], BF16, tag="w2", name=f"H_w2_{s}_{fb}")
                            nc.sync.dma_start(out=w2t[:], in_=w2[fb * 128:(fb + 1) * 128,
                                                               s * 512:(s + 1) * 512])
                            for tb in range(4):
                                nc.tensor.matmul(out=psY2[tb][:],
                                                 lhsT=hT_sb[:, fb, tb * 128:(tb + 1) * 128],
                                                 rhs=w2t[:], start=(fb == 0),
                                                 stop=(fb == NFB - 1), skip_group_check=True)
                        for tb in range(4):
                            if tb % 2 == 0:
                                nc.scalar.copy(y2_sb[:, tb, s * 512:(s + 1) * 512], psY2[tb][:])
                            else:
                                nc.vector.tensor_copy(y2_sb[:, tb, s * 512:(s + 1) * 512],
                                                      psY2[tb][:])
                for tb in range(4):
                    nc.vector.tensor_add(y2_sb[:, tb, :], y2_sb[:, tb, :], a2_sb[:, tb, :])
                    st = sb_ln.tile([P, 2, 6], F32, tag="st", name=f"H_st_{tb}")
                    nc.vector.bn_stats(out=st[:, 0, :], in_=y2_sb[:, tb, 0:512])
                    nc.vector.bn_stats(out=st[:, 1, :], in_=y2_sb[:, tb, 512:1024])
                    mv = sb_ln.tile([P, 2], F32, tag="mv", name=f"H_mv_{tb}")
                    nc.vector.bn_aggr(out=mv[:], in_=st[:])
                    nc.scalar.activation(out=mv[:, 1:2], in_=mv[:, 1:2], func=AF.Sqrt,
                                         bias=eps_t[:])
                    nc.vector.reciprocal(mv[:, 1:2], mv[:, 1:2])
                    osb = sb_ln.tile([P, D], F32, tag="osb", name=f"H_osb_{tb}")
                    nc.vector.tensor_scalar(out=osb[:], in0=y2_sb[:, tb, :],
                                            scalar1=mv[:, 0:1], scalar2=mv[:, 1:2],
                                            op0=ALU.subtract, op1=ALU.mult)
                    nc.vector.tensor_mul(osb[:], osb[:], g3[:])
                    nc.vector.tensor_add(osb[:], osb[:], b3[:])
                    nc.sync.dma_start(out=out[tb * 128:(tb + 1) * 128, :], in_=osb[:])

        for _rep in range(rep):
            emit_body(stop_after)

    nc.compile()
    return nc


_NC_CACHE = None


def _get_nc():
    global _NC_CACHE
    if _NC_CACHE is None:
        _NC_CACHE = build_kernel()
    return _NC_CACHE


def own_rows(c):
    """True row indices owned by rank c (within its batch), in local order."""
    return TOWN * c + np.arange(TOWN)


def make_in_maps(inputs):
    """Build the 8 per-core input dicts from the full problem inputs."""
    g = {k: np.asarray(v) for k, v in inputs.items()}
    la = g["lookahead_mask"]
    pm = g["padding_mask"]
    assert np.array_equal(la[0, 0], np.tril(np.ones((T, T), la.dtype))), \
        "kernel specialized for causal lookahead_mask"
    assert pm.min() == 1, "kernel specialized for all-ones padding_mask"

    r32 = round_fp32r
    # host-side bias folds (all fp64 for accuracy)
    sa_fold = (g["sa_bv"].astype(np.float64) @ g["sa_Wo"].astype(np.float64)
               + g["sa_bo"].astype(np.float64))                  # [D]
    cb = (g["ca_bv"].astype(np.float64) @ g["ca_Wo"].astype(np.float64)
          + g["ca_bo"].astype(np.float64))                       # [D]
    ln1_b = g["ln1_b"].astype(np.float64) + cb
    ln2_b = g["ln2_b"].astype(np.float64) + g["ff_b2"].astype(np.float64)
    ca_bq = g["ca_bq"].astype(np.float64) - cb @ g["ca_Wq"].astype(np.float64)
    b1_full = (g["ff_b1"].astype(np.float64)
               - g["ff_b2"].astype(np.float64) @ g["ff_W1"].astype(np.float64))

    qk_b_h = np.zeros((NC // 2, P, 2, 2), np.float32)
    for c in range(NC // 2):
        hsl = slice(DKL * c, DKL * (c + 1))
        for i, bias in enumerate((g["sa_bq"], g["sa_bk"])):
            qk_b_h[c, :, :, i] = np.asarray(bias)[hsl].reshape(2, 128).T.astype(np.float32)
    cab = np.zeros((P, 8, 2), np.float32)
    cab[:, :, 0] = ca_bq.reshape(8, 128).T.astype(np.float32)
    cab[:, :, 1] = np.asarray(g["ca_bk"]).reshape(8, 128).T.astype(np.float32)

    in_maps = []
    for r in range(NC):
        b, c = r // TPG, r % TPG
        hsl = slice(DKL * c, DKL * (c + 1))
        rows = own_rows(c)
        m = dict(
            xT=to_bf16(np.ascontiguousarray(g["x"][b].T)),
            x_rows=np.ascontiguousarray(
                g["x"][b][rows].astype(np.float64) + sa_fold).astype(np.float32),
            encT=to_bf16(np.ascontiguousarray(g["encoder_output"][b].T)),
            saq_w=to_bf16(g["sa_Wq"][:, hsl]), sak_w=to_bf16(g["sa_Wk"][:, hsl]),
            sav_w=to_bf16(g["sa_Wv"][:, hsl]),
            caq_w=to_bf16(g["ca_Wq"]),
            cak_w=to_bf16(g["ca_Wk"]), cav_w=to_bf16(g["ca_Wv"]),
            qk_b=qk_b_h[c], cab=cab,
            sao_w=to_bf16(g["sa_Wo"][hsl, :]), cao_w=to_bf16(g["ca_Wo"]),
            w1=to_bf16(g["ff_W1"]),
            b1=np.ascontiguousarray(
                b1_full.astype(np.float32).reshape(NFB, P).T),
            w2=to_bf16(g["ff_W2"]),
            ln_g=np.stack([g["ln1_g"], g["ln2_g"], g["ln3_g"]])[:, None].astype(np.float32),
            ln_b=np.stack([ln1_b.astype(np.float32), ln2_b.astype(np.float32),
                           g["ln3_b"]])[:, None].astype(np.float32),
        )
        in_maps.append(m)
    return in_maps


def assemble(results):
    outp = np.empty((B, T, D), np.float32)
    for r in range(NC):
        b, c = r // TPG, r % TPG
        outp[b][own_rows(c)] = results[r]["out"]
    return outp


def kernel(**inputs) -> np.ndarray:
    nc = _get_nc()
    in_maps = make_in_maps(inputs)
    res = run_bass_kernel_spmd(nc, in_maps, core_ids=list(range(NC)), trace=False)
    return assemble(res.results)
